# revision 49
# baseline (speedup 1.0000x reference)
"""Trainium2 Bass kernel for DeformationNetworkGraphConvolutionalFullRes.

Full (unsharded) inputs in, full output out. Data-parallel over the 4 meshes:
core m processes mesh m (cores 4-7 idle). Inside each core:

  - vert_align sampling is computed as (S @ F) @ W == S @ (F @ W): per feature
    map, F[C,HW] @ Wslice[C,128] -> G[HW,128] (tiny matmuls), then the sparse
    bilinear operator S (4 nonzeros/row) is applied as dense [128px, 512vert]
    blocks (built host-side from the vertex coordinates) streamed into the
    TensorEngine, accumulating over maps/pixel-tiles in PSUM. Vertices are
    pre-sorted by image cell so each 512-vertex chunk touches few pixel tiles.
  - Each GraphConv layer: h1 = x@W1 rows are written to HBM in a
    partition-major layout (full-bandwidth writes); messages h1[src] are
    pulled with dma_gather in dst-sorted edge order; the segmented sum over
    edges is done as one-hot matmuls accumulating in PSUM on top of
    h0 = x@W0 (+ rank-1 image-encoding term), then ReLU writes the
    transposed activations for the next layer directly.

Optimizations vs the original version:
  - everything streams in bf16 (activations, weights, feature maps, sampling
    blocks); PSUM accumulation stays fp32.
  - per-mesh degree-balanced vertex->tile packing (within each 512-vertex
    sampling chunk) minimizes edge-subchunk padding across the SPMD cores.
  - h1 rows live in DRAM partition-major so the per-layer write runs at full
    DMA bandwidth.
  - the scatter one-hots are built once (layer 0) and persist in SBUF.
  - the sampling schedule is ragged (no zero-block padding).
"""

import ml_dtypes
import numpy as np
from contextlib import ExitStack

import concourse.bass as bass
import concourse.tile as tile
from concourse import bacc, mybir
from concourse.bass_utils import run_bass_kernel_spmd

# ---------------- problem constants (hardcoded per spec) ----------------
B = 4
V = 10242
E_PER = 30720
HID = 128
MAPS = [(256, 56), (512, 28), (1024, 14), (2048, 7)]  # (C, H==W)
CH_OFF = [0, 256, 768, 1792, 3840]

VP = 10752            # padded vertex count: 21 chunks of 512 = 84 tiles of 128
NT = VP // 128        # 84 vertex tiles
NVCH = VP // 512      # 21 vertex chunks (sampling)
GT = 4                # dst tiles per gather group (group == sampling chunk)
NGRP = NT // GT       # 21 gather groups
G_PERS = 14           # groups with persistent one-hots (rest rebuilt per layer)
HB = 6                # h1 write batch (tiles, layer-0 prologue)

F32 = mybir.dt.float32
BF16 = mybir.dt.bfloat16
FP8 = mybir.dt.float8e4
I32 = mybir.dt.int32
I16 = mybir.dt.int16
AF = mybir.ActivationFunctionType
BF = ml_dtypes.bfloat16
F8 = ml_dtypes.float8_e4m3fn


def _corners(grid, W):
    """grid [V,2] in [-1,1] -> list of (pix_idx int32, weight f32) per corner."""
    x = (grid[:, 0] + 1.0) * 0.5 * (W - 1)
    y = (grid[:, 1] + 1.0) * 0.5 * (W - 1)
    x0f, y0f = np.floor(x), np.floor(y)
    wx1, wy1 = (x - x0f).astype(np.float32), (y - y0f).astype(np.float32)
    wx0, wy0 = 1.0 - wx1, 1.0 - wy1
    x0 = np.clip(x0f, 0, W - 1).astype(np.int64)
    x1 = np.clip(x0f + 1, 0, W - 1).astype(np.int64)
    y0 = np.clip(y0f, 0, W - 1).astype(np.int64)
    y1 = np.clip(y0f + 1, 0, W - 1).astype(np.int64)
    return [
        (y0 * W + x0, wy0 * wx0),
        (y0 * W + x1, wy0 * wx1),
        (y1 * W + x0, wy1 * wx0),
        (y1 * W + x1, wy1 * wx1),
    ]


def _balance_chunk(vids, degs, caps):
    """Assign the chunk's vertices to 4 tiles of 128 slots, packing each tile
    to at most caps[t] edge load where possible (caps are multiples of 128).
    Returns the vertex ids in new order (tile-by-tile)."""
    order = np.argsort(-degs, kind="stable")
    slots = [0, 0, 0, 0]
    loads = [0.0, 0.0, 0.0, 0.0]
    buckets = [[], [], [], []]
    for i in order:
        d = degs[i]
        # most-headroom tile that still fits under its cap
        best, best_room = -1, -1.0
        for t in range(4):
            room = caps[t] - loads[t]
            if slots[t] < 128 and room >= d and room > best_room:
                best, best_room = t, room
        if best < 0:  # cap bust: most headroom among tiles with free slots
            cands = [t for t in range(4) if slots[t] < 128]
            best = max(cands, key=lambda t: caps[t] - loads[t])
        buckets[best].append(vids[i])
        slots[best] += 1
        loads[best] += d
    out = []
    for t in range(4):
        out.extend(buckets[t])
    return np.array(out, dtype=np.int64)


def _prep(inputs):
    """Host-side restructuring: sorting, balancing, padding, index tables,
    sparse-operator blocks. Returns (cfg, per_core_aux_list, post)."""
    feats = [inputs["feat1"], inputs["feat2"], inputs["feat3"], inputs["feat4"]]
    av = np.asarray(inputs["aligned_verts"], np.float32)
    verts = np.asarray(inputs["verts_packed"], np.float32)
    enc = np.asarray(inputs["image_enc"], np.float32)
    edges = np.asarray(inputs["edges"], np.int64)

    for bn in ["bottleneck_b", "g0_b0", "g0_b1", "off_b"]:
        assert not np.any(np.asarray(inputs[bn])), f"{bn} nonzero: unsupported"
    assert not np.any(np.asarray(inputs["gb0"])) and not np.any(
        np.asarray(inputs["gb1"])
    ), "gb nonzero: unsupported"

    # per-mesh vertex sort (by finest-map cell) + degree balance ------------
    # chunk membership is fixed by the cell sort; only the tile assignment
    # within each 512-vertex chunk is rebalanced.  The per-chunk subchunk
    # budget K_c comes from the max-over-meshes chunk load, spread evenly
    # over the 4 tiles (shared SPMD structure).
    sigmas0, degs_all = [], []
    for m in range(B):
        grid = av[m, :, :2]
        cs = _corners(grid, MAPS[0][1])
        key = cs[0][0]  # y0*56+x0 of map 0
        sigmas0.append(np.argsort(key, kind="stable"))
        e = edges[m * E_PER:(m + 1) * E_PER] - m * V
        degs_all.append(np.bincount(np.concatenate([e[:, 0], e[:, 1]]),
                                    minlength=V).astype(np.float64))
    caps_c = []
    for c in range(NVCH):
        lo, hi = c * 512, min((c + 1) * 512, V)
        tmax = 0.0
        if lo < V:
            for m in range(B):
                tmax = max(tmax, degs_all[m][sigmas0[m][lo:hi]].sum())
        kc = max(4, int(-(-tmax // 128)))
        base, rem = kc // 4, kc % 4
        caps_c.append([128 * (base + (1 if i < rem else 0)) for i in range(4)])

    sigmas, invs, corners_all = [], [], []
    for m in range(B):
        grid = av[m, :, :2]
        sigma = sigmas0[m]
        deg = degs_all[m]
        balanced = np.empty_like(sigma)
        for c in range(NVCH):
            lo, hi = c * 512, min((c + 1) * 512, V)
            if lo >= V:
                break
            vids = sigma[lo:hi]
            balanced[lo:hi] = _balance_chunk(vids, deg[vids], caps_c[c])
        sigma = balanced
        inv = np.empty(V, np.int64)
        inv[sigma] = np.arange(V)
        sigmas.append(sigma)
        invs.append(inv)
        corners_all.append(
            [[(pix[sigma], w[sigma]) for (pix, w) in _corners(grid, Wm)]
             for (_, Wm) in MAPS]
        )

    # sampling schedule: per (chunk, map) the union over meshes of touched
    # pixel tiles; ragged (no padding) ------------------------------------
    ntile_map = [(Wm * Wm + 127) // 128 for (_, Wm) in MAPS]
    g_off = np.cumsum([0] + ntile_map)  # global G-tile offsets
    sched = []  # sched[mi][c] = list of pixel-tile indices
    for mi in range(4):
        per_c = []
        for c in range(NVCH):
            lo, hi = c * 512, min((c + 1) * 512, V)
            tiles = set()
            if lo < V:
                for m in range(B):
                    for (pix, _w) in corners_all[m][mi]:
                        pc = pix[lo:hi] // 128
                        tiles.update(np.unique(pc).tolist())
            per_c.append(sorted(tiles) if tiles else [0])
        sched.append(per_c)
    npc = [sum(len(sched[mi][c]) for mi in range(4)) for c in range(NVCH)]
    npc_off = np.concatenate([[0], np.cumsum(npc)]).astype(int)
    npair = int(npc_off[-1])
    max_npc = max(npc)

    # graph structure ------------------------------------------------------
    # directed edges sorted by dst, grouped per dst tile; per-tile subchunk
    # count = max over meshes (keeps one SPMD instruction stream)
    ecnts = []
    esorted = []
    for m in range(B):
        e = edges[m * E_PER:(m + 1) * E_PER] - m * V
        a = invs[m][e[:, 0]]
        b = invs[m][e[:, 1]]
        dst = np.concatenate([a, b])
        src = np.concatenate([b, a])
        order = np.lexsort((src, dst))
        esorted.append((dst[order], src[order]))
        ecnts.append(np.bincount(dst // 128, minlength=NT))
    nsub_t = np.maximum(1, -(-np.stack(ecnts).max(axis=0) // 128))  # [NT]
    assert nsub_t.max() <= 8
    sub_off = np.concatenate([[0], np.cumsum(nsub_t)]).astype(int)
    tot_sub = int(sub_off[-1])
    sub_g_max = int(max(sub_off[(g + 1) * GT] - sub_off[g * GT]
                        for g in range(NGRP)))

    per_core = []
    for m in range(B):
        dst, src = esorted[m]
        counts = ecnts[m]
        src_slots = np.zeros((tot_sub, 128), np.int64)
        dl_slots = np.full((tot_sub, 128), -1, np.int32)
        pos = 0
        for t in range(NT):
            cnt = counts[t]
            so = sub_off[t] * 128
            src_slots.reshape(-1)[so:so + cnt] = src[pos:pos + cnt]
            dl_slots.reshape(-1)[so:so + cnt] = dst[pos:pos + cnt] - t * 128
            pos += cnt
        # remap src vertex id -> partition-major h1d row: (v%128)*NT + v//128
        src_lin = ((src_slots % 128) * NT + src_slots // 128).reshape(-1)
        # wrapped int16 for dma_gather: idx i at (i%16, i//16), replicated 8x
        srcw = np.tile(src_lin.reshape(-1, 16).T, (8, 1)).astype(np.int16)
        # dst_local per (partition, subchunk)
        dl = dl_slots.reshape(tot_sub, 128).T.copy().astype(ml_dtypes.bfloat16)

        # sampling blocks (ragged, chunk-major) ----------------------------
        wsc = np.zeros((npair, 128, 512), np.float32)
        pi = 0
        for c in range(NVCH):
            lo, hi = c * 512, min((c + 1) * 512, V)
            for mi in range(4):
                for t in sched[mi][c]:
                    blk = wsc[pi]
                    if lo < V:
                        for (pix, w) in corners_all[m][mi]:
                            px = pix[lo:hi]
                            sel = (px >= t * 128) & (px < (t + 1) * 128)
                            jj = np.nonzero(sel)[0]
                            np.add.at(blk, (px[jj] - t * 128, jj), w[lo:hi][jj])
                    pi += 1
        assert pi == npair

        vt = np.zeros((3, VP), np.float32)
        vt[:, :V] = verts[m * V:(m + 1) * V][sigmas[m]].T

        aux = {
            "f1": np.ascontiguousarray(feats[0][m].reshape(256, -1)).astype(BF),
            "f2": np.ascontiguousarray(feats[1][m].reshape(512, -1)).astype(BF),
            "f3": np.ascontiguousarray(feats[2][m].reshape(1024, -1)).astype(BF),
            "f4": np.ascontiguousarray(feats[3][m].reshape(2048, -1)).astype(BF),
            "bw": np.ascontiguousarray(
                np.asarray(inputs["bottleneck_w"], np.float32)
                .reshape(30, 128, HID).transpose(1, 0, 2)
                .reshape(128, 30 * HID)).astype(BF),
            "wsc": wsc.reshape(npair * 128, 512).astype(F8),
            "srcw": np.ascontiguousarray(srcw),
            "dstloc": np.ascontiguousarray(dl),
            "iota": np.tile(np.arange(128, dtype=BF), (128, 1)),
            "vertsT": vt.astype(BF),
            "encc": enc[m].reshape(2, 128).T.copy().astype(BF),  # [128, 2]
            "g0w0m": np.asarray(inputs["g0_w0"][:128], np.float32).astype(BF),
            "g0w0v": np.asarray(inputs["g0_w0"][128:131], np.float32).astype(BF),
            "g0w0e": np.ascontiguousarray(
                np.asarray(inputs["g0_w0"][131:387], np.float32)).astype(BF),
            "g0w1m": np.asarray(inputs["g0_w1"][:128], np.float32).astype(BF),
            "g0w1v": np.asarray(inputs["g0_w1"][128:131], np.float32).astype(BF),
            "g0w1e": np.ascontiguousarray(
                np.asarray(inputs["g0_w1"][131:387], np.float32)).astype(BF),
            "gw0": np.ascontiguousarray(
                np.asarray(inputs["gw0"], np.float32).transpose(1, 0, 2)
                .reshape(128, 7 * 128)).astype(BF),
            "gw1": np.ascontiguousarray(
                np.asarray(inputs["gw1"], np.float32).transpose(1, 0, 2)
                .reshape(128, 7 * 128)).astype(BF),
            "offw": np.asarray(inputs["off_w"], np.float32).astype(BF),
        }
        per_core.append(aux)

    cfg = {"sched": sched, "npc": npc, "npc_off": npc_off.tolist(),
           "npair": npair, "max_npc": max_npc,
           "g_off": g_off.tolist(), "ntile_map": ntile_map,
           "nsub_t": nsub_t.tolist(), "sub_off": sub_off.tolist(),
           "tot_sub": tot_sub, "sub_g_max": sub_g_max}
    post = {"sigmas": sigmas}
    return cfg, per_core, post


def _build(cfg, shapes, dump=None, nlayers=8, repeat=1):
    """Build the SPMD Bass program (same instruction stream for all cores)."""
    nc = bacc.Bacc("TRN2", target_bir_lowering=False, debug=False, num_devices=B)
    ap = {}
    for name, arr in shapes.items():
        ap[name] = nc.dram_tensor(
            name, list(arr.shape), mybir.dt.from_np(arr.dtype),
            kind="ExternalInput").ap()
    out = nc.dram_tensor("out", [3, VP], F32, kind="ExternalOutput").ap()
    xdump = (nc.dram_tensor("xdump", [128, VP], F32, kind="ExternalOutput").ap()
             if dump else None)
    h1d2 = [nc.dram_tensor("h1da", [VP, HID], BF16).ap(),
            nc.dram_tensor("h1db", [VP, HID], BF16).ap()]

    sched = cfg["sched"]
    npc = cfg["npc"]
    npc_off = cfg["npc_off"]
    max_npc = cfg["max_npc"]
    g_off = cfg["g_off"]
    ntile_map = cfg["ntile_map"]
    NGT = g_off[4]  # total G tiles
    tot_sub = cfg["tot_sub"]
    nsub_t = cfg["nsub_t"]
    sub_off = cfg["sub_off"]
    sub_g_max = cfg["sub_g_max"]

    with tile.TileContext(nc) as tc, ExitStack() as ctx:
        # ---------------- persistent pools ----------------
        s_pers = sub_off[G_PERS * GT]  # persistent one-hot subchunks
        pp = ctx.enter_context(tc.tile_pool(name="pers", bufs=1))
        xa = pp.tile([128, VP], BF16, tag="xa")
        xb = pp.tile([128, VP], BF16, tag="xb")
        oh_pers = pp.tile([128, s_pers, 128], BF16, tag="ohp")
        srcw_t = pp.tile([128, tot_sub * 8], I16, tag="srcw")
        dstloc_t = pp.tile([128, tot_sub, 1], BF16, tag="dstloc")
        iota_t = pp.tile([128, 1, 128], BF16, tag="iota")
        w0_t = pp.tile([128, 7 * 128], BF16, tag="w0")
        w1_t = pp.tile([128, 7 * 128], BF16, tag="w1")
        g0_t = pp.tile([128, 6 * 128], BF16, tag="g0")  # w0m,w1m,w0e(2),w1e(2)
        g0v_t = pp.tile([3, 256], BF16, tag="g0v")      # w0v, w1v
        offw_t = pp.tile([128, 3], BF16, tag="offw")
        ones_t = pp.tile([1, GT * 128], BF16, tag="ones")
        erow_t = pp.tile([1, 256], BF16, tag="erow")    # e0row, e1row
        encc_t = pp.tile([128, 2], BF16, tag="encc")

        nc.vector.memset(ones_t[:], 1.0)

        psA = ctx.enter_context(tc.tile_pool(name="psA", bufs=2, space="PSUM"))

        def _load_g0():
            """Layer-0 weight loads + enc rank-1 rows; issued after the first
            feature-map DMAs so they don't delay the sampling pipeline."""
            nc.sync.dma_start(g0_t[:, 0:128], ap["g0w0m"][:])
            nc.sync.dma_start(g0_t[:, 128:256], ap["g0w1m"][:])
            nc.sync.dma_start(
                g0_t[:, 256:512].rearrange("p (c h) -> p c h", h=128),
                ap["g0w0e"].rearrange("(c p) h -> p c h", p=128))
            nc.sync.dma_start(
                g0_t[:, 512:768].rearrange("p (c h) -> p c h", h=128),
                ap["g0w1e"].rearrange("(c p) h -> p c h", p=128))
            nc.sync.dma_start(g0v_t[:, 0:128], ap["g0w0v"][:])
            nc.sync.dma_start(g0v_t[:, 128:256], ap["g0w1v"][:])
            nc.sync.dma_start(offw_t[:], ap["offw"][:])
            nc.sync.dma_start(encc_t[:], ap["encc"][:])
            # enc rank-1 rows: e{0,1} = g0_w{0,1}[131:387].T @ enc -> [1,128]
            for k in range(2):
                pe = psA.tile([1, 128], F32, tag="p1")
                for cchunk in range(2):
                    nc.tensor.matmul(
                        out=pe[:],
                        lhsT=encc_t[:, cchunk:cchunk + 1],
                        rhs=g0_t[:, 256 + k * 256 + cchunk * 128:
                                 256 + k * 256 + cchunk * 128 + 128],
                        start=(cchunk == 0), stop=(cchunk == 1))
                nc.scalar.activation(erow_t[:, k * 128:(k + 1) * 128], pe[:],
                                     AF.Copy)

        def _sampling(sctx):
            """Phase 1: vert_align sampling -> xa (bf16 columns).  The layer-0
            h1 rows are produced chunk-by-chunk right after each ReLU so the
            first gathers can start as soon as sampling ends."""
            sp = sctx.enter_context(tc.tile_pool(name="samp", bufs=1))
            spfm = sctx.enter_context(tc.tile_pool(name="sampfm", bufs=2))
            spf = sctx.enter_context(tc.tile_pool(name="sampf", bufs=2))
            spw = sctx.enter_context(tc.tile_pool(name="sampw", bufs=4))
            sph = sctx.enter_context(tc.tile_pool(name="samph", bufs=2))
            spp1 = sctx.enter_context(tc.tile_pool(name="samppsum1", bufs=2,
                                                   space="PSUM"))
            spp2 = sctx.enter_context(tc.tile_pool(name="samppsum2", bufs=2,
                                                   space="PSUM"))
            g_sb = sp.tile([128, NGT * 128], BF16, tag="gsb")

            def _load_map(mi):
                C, Wm = MAPS[mi]
                ncc = C // 128
                bw_t = spf.tile([128, 16 * 128], BF16, tag="bw")
                nc.sync.dma_start(
                    bw_t[:, :ncc * 128],
                    ap["bw"][:, CH_OFF[mi]:CH_OFF[mi] + ncc * 128])
                fm_t = spfm.tile([128, 2 * 3136], BF16, tag="fm")
                nc.sync.dma_start(
                    fm_t[:, :ncc * Wm * Wm].rearrange(
                        "p (c hw) -> p c hw", c=ncc),
                    ap[f"f{mi+1}"].rearrange("(c p) hw -> p c hw", p=128))
                return fm_t, bw_t

            nxt_ld = _load_map(0)
            _load_g0()
            for mi, (C, Wm) in enumerate(MAPS):
                HW = Wm * Wm
                ncc = C // 128
                fm_t, bw_t = nxt_ld
                if mi + 1 < 4:
                    nxt_ld = _load_map(mi + 1)
                for t in range(ntile_map[mi]):
                    p0 = t * 128
                    pcnt = min(128, HW - p0)
                    pg = psA.tile([128, 128], F32, tag="p1")
                    for cc in range(ncc):
                        nc.tensor.matmul(
                            out=pg[:pcnt, :],
                            lhsT=fm_t[:, cc * HW + p0:cc * HW + p0 + pcnt],
                            rhs=bw_t[:, cc * 128:cc * 128 + 128],
                            start=(cc == 0), stop=(cc == ncc - 1))
                    gt = g_off[mi] + t
                    nc.scalar.activation(
                        g_sb[:pcnt, gt * 128:gt * 128 + 128], pg[:pcnt, :],
                        AF.Copy)

            for c in range(NVCH):
                ps = spp1.tile([128, 512], F32, tag="ps")
                pairs_c = []
                for mi in range(4):
                    for t in sched[mi][c]:
                        pairs_c.append((mi, t))
                assert len(pairs_c) == npc[c]
                half = (max_npc + 1) // 2
                nh = (npc[c] + half - 1) // half
                wts = []
                for hb in range(nh):
                    k0, k1 = hb * half, min((hb + 1) * half, npc[c])
                    wt = spw.tile([128, half, 512], FP8, tag="wsc")
                    nc.sync.dma_start(
                        wt[:, :k1 - k0, :],
                        ap["wsc"].rearrange("(k p) h -> p k h", p=128)
                        [:, npc_off[c] + k0:npc_off[c] + k1, :])
                    wts.append(wt)
                for k, (mi, t) in enumerate(pairs_c):
                    HW = MAPS[mi][1] ** 2
                    pcnt = min(128, HW - t * 128)
                    gt = g_off[mi] + t
                    nc.tensor.matmul(
                        out=ps[:],
                        lhsT=g_sb[:pcnt, gt * 128:gt * 128 + 128],
                        rhs=wts[k // half][:pcnt, k % half, :],
                        start=(k == 0), stop=(k == len(pairs_c) - 1))
                nc.scalar.activation(xa[:, c * 512:(c + 1) * 512], ps[:],
                                     AF.Relu)
                # layer-0 h1 rows for this chunk's 4 tiles
                vv = sph.tile([3, 512], BF16, tag="vt")
                nc.sync.dma_start(vv[:],
                                  ap["vertsT"][:, c * 512:(c + 1) * 512])
                ph4 = spp2.tile([128, 512], F32, tag="ph4")
                hstc = sph.tile([128, 512], BF16, tag="hstc")
                for ti in range(4):
                    t = 4 * c + ti
                    sl = slice(ti * 128, (ti + 1) * 128)
                    nc.tensor.matmul(
                        out=ph4[:, sl], lhsT=xa[:, t * 128:(t + 1) * 128],
                        rhs=g0_t[:, 128:256], start=True, stop=False)
                    nc.tensor.matmul(
                        out=ph4[:, sl], lhsT=vv[:, sl],
                        rhs=g0v_t[:, 128:256], start=False, stop=False)
                    nc.tensor.matmul(
                        out=ph4[:, sl], lhsT=ones_t[:, 0:128],
                        rhs=erow_t[:, 128:256], start=False, stop=True)
                nc.scalar.activation(hstc[:], ph4[:], AF.Copy)
                # Pool-issued so a write waiting on compute never head-of-line
                # blocks the SP queue's wsc prefetch stream.
                h1_writes.append(nc.gpsimd.dma_start(
                    h1d2[0].rearrange("(p n) c -> p n c", p=128)
                    [:, c * 4:(c + 1) * 4, :],
                    hstc[:].rearrange("p (n c) -> p n c", c=128)))

        h1_writes = []
        with ExitStack() as sctx:
            _sampling(sctx)

        # bulky graph-structure/weight loads land during the sampling phase
        nc.sync.dma_start(srcw_t[:], ap["srcw"][:])
        nc.sync.dma_start(
            dstloc_t[:], ap["dstloc"].rearrange("p (s o) -> p s o", o=1))
        nc.sync.dma_start(iota_t[:].rearrange("p o d -> p (o d)"),
                          ap["iota"][:])
        nc.sync.dma_start(w0_t[:], ap["gw0"][:])
        nc.sync.dma_start(w1_t[:], ap["gw1"][:])

        # ---------------- phase 2: graph conv layers ----------------
        lp = ctx.enter_context(tc.tile_pool(name="lay", bufs=3))
        lpo = ctx.enter_context(tc.tile_pool(name="layoh", bufs=2))
        lph = ctx.enter_context(tc.tile_pool(name="layh", bufs=2))
        lpv = ctx.enter_context(tc.tile_pool(name="layv", bufs=1))
        psx = ctx.enter_context(tc.tile_pool(name="psumx", bufs=2, space="PSUM"))
        psB = ctx.enter_context(tc.tile_pool(name="psumo", bufs=1, space="PSUM"))

        def _layers(first_rep, last_rep, h1_writes):
            cur, nxt = xa, xb
            if not first_rep:
                # prologue: recompute layer-0 h1 rows (repeat mode only)
                h1_writes = []
                for g in range(NGRP):
                    hst = lph.tile([128, GT * 128], BF16, tag="hstg")
                    vv = lpv.tile([3, GT * 128], BF16, tag="vt")
                    nc.sync.dma_start(
                        vv[:], ap["vertsT"][:, g * 512:(g + 1) * 512])
                    ph4 = psx.tile([128, 512], F32, tag="ph4")
                    for ti in range(GT):
                        t = g * GT + ti
                        sl = slice(ti * 128, (ti + 1) * 128)
                        nc.tensor.matmul(
                            out=ph4[:, sl], lhsT=cur[:, t * 128:(t + 1) * 128],
                            rhs=g0_t[:, 128:256], start=True, stop=False)
                        nc.tensor.matmul(
                            out=ph4[:, sl], lhsT=vv[:, sl],
                            rhs=g0v_t[:, 128:256], start=False, stop=False)
                        nc.tensor.matmul(
                            out=ph4[:, sl], lhsT=ones_t[:, 0:128],
                            rhs=erow_t[:, 128:256], start=False, stop=True)
                    nc.scalar.activation(hst[:], ph4[:], AF.Copy)
                    h1_writes.append(nc.sync.dma_start(
                        h1d2[0].rearrange("(p n) c -> p n c", p=128)
                        [:, g * GT:(g + 1) * GT, :],
                        hst[:].rearrange("p (n c) -> p n c", c=128)))

            for l in range(nlayers):
                h1d = h1d2[l % 2]
                h1d_nxt = h1d2[(l + 1) % 2]
                next_writes = []

                # gather groups + scatter matmuls; h1 rows for layer l+1 are
                # produced group-by-group right after each ReLU so the next
                # layer's gathers can start almost immediately.
                for g in range(NGRP):
                    s0 = sub_off[g * GT]
                    s1 = sub_off[min((g + 1) * GT, NT)]
                    ng = s1 - s0
                    msg = lp.tile([128, sub_g_max, 128], BF16, tag="msg")
                    gi = nc.gpsimd.dma_gather(
                        out_ap=msg[:, :ng, :],
                        in_ap=h1d[:],
                        idxs_ap=srcw_t[:, s0 * 8:s1 * 8],
                        num_idxs=ng * 128,
                        num_idxs_reg=ng * 128,
                        elem_size=HID,
                        single_packet=False,
                    )
                    for wi in h1_writes:
                        tile.add_dep_helper(gi.ins, wi.ins,
                                            reason="h1 RAW: gather after write")
                    if g < G_PERS:
                        oh_t, so = oh_pers, 0
                        if l == 0 and first_rep:
                            # build the persistent one-hots (layer-invariant)
                            nc.vector.tensor_tensor(
                                out=oh_pers[:, s0:s1, :],
                                in0=dstloc_t[:, s0:s1, :]
                                .to_broadcast([128, ng, 128]),
                                in1=iota_t[:].to_broadcast([128, ng, 128]),
                                op=mybir.AluOpType.is_equal)
                    else:
                        oh_t = lpo.tile([128, sub_g_max, 128], BF16, tag="oht")
                        so = s0
                        nc.vector.tensor_tensor(
                            out=oh_t[:, :ng, :],
                            in0=dstloc_t[:, s0:s1, :]
                            .to_broadcast([128, ng, 128]),
                            in1=iota_t[:].to_broadcast([128, ng, 128]),
                            op=mybir.AluOpType.is_equal)
                    if l == 0:
                        vv2 = lpv.tile([3, GT * 128], BF16, tag="vt2")
                        nc.sync.dma_start(
                            vv2[:],
                            ap["vertsT"][:, g * GT * 128:(g + 1) * GT * 128])
                    W = GT * 128
                    px = psx.tile([128, W], F32, tag="px")
                    if l == 0:
                        nc.tensor.matmul(
                            out=px[:], lhsT=g0_t[:, 0:128],
                            rhs=cur[:, g * W:(g + 1) * W],
                            start=True, stop=False)
                        nc.tensor.matmul(
                            out=px[:], lhsT=g0v_t[:, 0:128],
                            rhs=vv2[:], start=False, stop=False)
                        nc.tensor.matmul(
                            out=px[:], lhsT=erow_t[:, 0:128],
                            rhs=ones_t[:], start=False, stop=False)
                    else:
                        nc.tensor.matmul(
                            out=px[:], lhsT=w0_t[:, (l - 1) * 128:l * 128],
                            rhs=cur[:, g * W:(g + 1) * W],
                            start=True, stop=False)
                    for ti in range(GT):
                        t = g * GT + ti
                        nst = nsub_t[t]
                        for j in range(nst):
                            s = sub_off[t] - s0 + j
                            nc.tensor.matmul(
                                out=px[:, ti * 128:(ti + 1) * 128],
                                lhsT=msg[:, s, :],
                                rhs=oh_t[:, sub_off[t] + j - so, :],
                                start=False,
                                stop=(ti == GT - 1 and j == nst - 1),
                                skip_group_check=True)
                    nc.scalar.activation(nxt[:, g * W:(g + 1) * W], px[:],
                                         AF.Relu)
                    if l == nlayers - 1 and last_rep:
                        # delta_v for this group: off_w.T @ x cols -> [3, 512]
                        po = psB.tile([3, GT * 128], F32, tag="po")
                        nc.tensor.matmul(
                            out=po[:], lhsT=offw_t[:],
                            rhs=nxt[:, g * W:(g + 1) * W],
                            start=True, stop=True)
                        ost = lph.tile([3, GT * 128], F32, tag="ost")
                        nc.scalar.activation(ost[:], po[:], AF.Copy)
                        nc.sync.dma_start(out[:, g * W:(g + 1) * W], ost[:])
                    if l + 1 < nlayers:
                        # h1 rows for layer l+1 on this group's tiles
                        hst = lph.tile([128, GT * 128], BF16, tag="hstg")
                        ph4 = psx.tile([128, 512], F32, tag="ph4")
                        for ti in range(GT):
                            t = g * GT + ti
                            nc.tensor.matmul(
                                out=ph4[:, ti * 128:(ti + 1) * 128],
                                lhsT=nxt[:, t * 128:(t + 1) * 128],
                                rhs=w1_t[:, l * 128:(l + 1) * 128],
                                start=True, stop=True)
                        nc.scalar.activation(hst[:], ph4[:], AF.Copy)
                        next_writes.append(nc.sync.dma_start(
                            h1d_nxt.rearrange("(p n) c -> p n c", p=128)
                            [:, g * GT:(g + 1) * GT, :],
                            hst[:].rearrange("p (n c) -> p n c", c=128)))
                h1_writes = next_writes
                cur, nxt = nxt, cur

        for _rep in range(repeat):
            _layers(_rep == 0, _rep == repeat - 1, h1_writes)
        cur = xa if nlayers % 2 == 0 else xb

        if xdump is not None:
            nc.sync.dma_start(xdump[:], cur[:])

        if nlayers == 0:
            # output straight from the sampled activations (debug path)
            for g in range(NGRP):
                po = psB.tile([3, GT * 128], F32, tag="po")
                nc.tensor.matmul(
                    out=po[:], lhsT=offw_t[:],
                    rhs=cur[:, g * 512:(g + 1) * 512], start=True, stop=True)
                ost = lph.tile([3, GT * 128], F32, tag="ost")
                nc.scalar.activation(ost[:], po[:], AF.Copy)
                nc.sync.dma_start(out[:, g * 512:(g + 1) * 512], ost[:])

    nc.compile()
    return nc


_CACHE = {}


def kernel(**inputs) -> np.ndarray:
    cfg, per_core, post = _prep(inputs)
    key = (cfg["npair"], tuple(cfg["npc"]), cfg["tot_sub"],
           tuple(cfg["nsub_t"]))
    if key not in _CACHE:
        _CACHE[key] = _build(cfg, per_core[0])
    nc = _CACHE[key]
    res = run_bass_kernel_spmd(nc, per_core, list(range(B)))
    outs = np.empty((B, V, 3), np.float32)
    for m in range(B):
        rows = np.ascontiguousarray(res.results[m]["out"].T)[:V]
        outs[m][post["sigmas"][m]] = rows
    return outs.reshape(B * V, 3)


if __name__ == "__main__":
    pass


# revision 55
# speedup vs baseline: 1.0124x; 1.0124x over previous
"""Trainium2 Bass kernel for DeformationNetworkGraphConvolutionalFullRes.

Full (unsharded) inputs in, full output out. Data-parallel over the 4 meshes:
core m processes mesh m (cores 4-7 idle). Inside each core:

  - vert_align sampling is computed as (S @ F) @ W == S @ (F @ W): per feature
    map, F[C,HW] @ Wslice[C,128] -> G[HW,128] (tiny matmuls), then the sparse
    bilinear operator S (4 nonzeros/row) is applied as dense [128px, 512vert]
    blocks (built host-side from the vertex coordinates) streamed into the
    TensorEngine, accumulating over maps/pixel-tiles in PSUM. Vertices are
    pre-sorted by image cell so each 512-vertex chunk touches few pixel tiles.
  - Each GraphConv layer: h1 = x@W1 rows are written to HBM in a
    partition-major layout (full-bandwidth writes); messages h1[src] are
    pulled with dma_gather in dst-sorted edge order; the segmented sum over
    edges is done as one-hot matmuls accumulating in PSUM on top of
    h0 = x@W0 (+ rank-1 image-encoding term), then ReLU writes the
    transposed activations for the next layer directly.

Optimizations vs the original version:
  - everything streams in bf16 (activations, weights, feature maps, sampling
    blocks); PSUM accumulation stays fp32.
  - per-mesh degree-balanced vertex->tile packing (within each 512-vertex
    sampling chunk) minimizes edge-subchunk padding across the SPMD cores.
  - h1 rows live in DRAM partition-major so the per-layer write runs at full
    DMA bandwidth.
  - the scatter one-hots are built once (layer 0) and persist in SBUF.
  - the sampling schedule is ragged (no zero-block padding).
"""

import ml_dtypes
import numpy as np
from contextlib import ExitStack

import concourse.bass as bass
import concourse.tile as tile
from concourse import bacc, mybir
from concourse.bass_utils import run_bass_kernel_spmd

# ---------------- problem constants (hardcoded per spec) ----------------
B = 4
V = 10242
E_PER = 30720
HID = 128
MAPS = [(256, 56), (512, 28), (1024, 14), (2048, 7)]  # (C, H==W)
CH_OFF = [0, 256, 768, 1792, 3840]

VP = 10752            # padded vertex count: 21 chunks of 512 = 84 tiles of 128
NT = VP // 128        # 84 vertex tiles
NVCH = VP // 512      # 21 vertex chunks (sampling)
GT = 4                # dst tiles per gather group (group == sampling chunk)
NGRP = NT // GT       # 21 gather groups
G_PERS = 14           # groups with persistent one-hots (rest rebuilt per layer)
HB = 6                # h1 write batch (tiles, layer-0 prologue)

F32 = mybir.dt.float32
BF16 = mybir.dt.bfloat16
FP8 = mybir.dt.float8e4
I32 = mybir.dt.int32
I16 = mybir.dt.int16
AF = mybir.ActivationFunctionType
BF = ml_dtypes.bfloat16
F8 = ml_dtypes.float8_e4m3fn


def _corners(grid, W):
    """grid [V,2] in [-1,1] -> list of (pix_idx int32, weight f32) per corner."""
    x = (grid[:, 0] + 1.0) * 0.5 * (W - 1)
    y = (grid[:, 1] + 1.0) * 0.5 * (W - 1)
    x0f, y0f = np.floor(x), np.floor(y)
    wx1, wy1 = (x - x0f).astype(np.float32), (y - y0f).astype(np.float32)
    wx0, wy0 = 1.0 - wx1, 1.0 - wy1
    x0 = np.clip(x0f, 0, W - 1).astype(np.int64)
    x1 = np.clip(x0f + 1, 0, W - 1).astype(np.int64)
    y0 = np.clip(y0f, 0, W - 1).astype(np.int64)
    y1 = np.clip(y0f + 1, 0, W - 1).astype(np.int64)
    return [
        (y0 * W + x0, wy0 * wx0),
        (y0 * W + x1, wy0 * wx1),
        (y1 * W + x0, wy1 * wx0),
        (y1 * W + x1, wy1 * wx1),
    ]


def _balance_chunk(vids, degs, caps):
    """Assign the chunk's vertices to 4 tiles of 128 slots, packing each tile
    to at most caps[t] edge load where possible (caps are multiples of 128).
    Returns the vertex ids in new order (tile-by-tile)."""
    order = np.argsort(-degs, kind="stable")
    slots = [0, 0, 0, 0]
    loads = [0.0, 0.0, 0.0, 0.0]
    buckets = [[], [], [], []]
    for i in order:
        d = degs[i]
        # most-headroom tile that still fits under its cap
        best, best_room = -1, -1.0
        for t in range(4):
            room = caps[t] - loads[t]
            if slots[t] < 128 and room >= d and room > best_room:
                best, best_room = t, room
        if best < 0:  # cap bust: most headroom among tiles with free slots
            cands = [t for t in range(4) if slots[t] < 128]
            best = max(cands, key=lambda t: caps[t] - loads[t])
        buckets[best].append(vids[i])
        slots[best] += 1
        loads[best] += d
    out = []
    for t in range(4):
        out.extend(buckets[t])
    return np.array(out, dtype=np.int64)


def _prep(inputs):
    """Host-side restructuring: sorting, balancing, padding, index tables,
    sparse-operator blocks. Returns (cfg, per_core_aux_list, post)."""
    feats = [inputs["feat1"], inputs["feat2"], inputs["feat3"], inputs["feat4"]]
    av = np.asarray(inputs["aligned_verts"], np.float32)
    verts = np.asarray(inputs["verts_packed"], np.float32)
    enc = np.asarray(inputs["image_enc"], np.float32)
    edges = np.asarray(inputs["edges"], np.int64)

    for bn in ["bottleneck_b", "g0_b0", "g0_b1", "off_b"]:
        assert not np.any(np.asarray(inputs[bn])), f"{bn} nonzero: unsupported"
    assert not np.any(np.asarray(inputs["gb0"])) and not np.any(
        np.asarray(inputs["gb1"])
    ), "gb nonzero: unsupported"

    # per-mesh vertex sort (by finest-map cell) + degree balance ------------
    # chunk membership is fixed by the cell sort; only the tile assignment
    # within each 512-vertex chunk is rebalanced.  The per-chunk subchunk
    # budget K_c comes from the max-over-meshes chunk load, spread evenly
    # over the 4 tiles (shared SPMD structure).
    sigmas0, degs_all = [], []
    for m in range(B):
        grid = av[m, :, :2]
        cs = _corners(grid, MAPS[0][1])
        key = cs[0][0]  # y0*56+x0 of map 0
        sigmas0.append(np.argsort(key, kind="stable"))
        e = edges[m * E_PER:(m + 1) * E_PER] - m * V
        degs_all.append(np.bincount(np.concatenate([e[:, 0], e[:, 1]]),
                                    minlength=V).astype(np.float64))
    caps_c = []
    for c in range(NVCH):
        lo, hi = c * 512, min((c + 1) * 512, V)
        tmax = 0.0
        if lo < V:
            for m in range(B):
                tmax = max(tmax, degs_all[m][sigmas0[m][lo:hi]].sum())
        kc = max(4, int(-(-tmax // 128)))
        base, rem = kc // 4, kc % 4
        caps_c.append([128 * (base + (1 if i < rem else 0)) for i in range(4)])

    sigmas, invs, corners_all = [], [], []
    for m in range(B):
        grid = av[m, :, :2]
        sigma = sigmas0[m]
        deg = degs_all[m]
        balanced = np.empty_like(sigma)
        for c in range(NVCH):
            lo, hi = c * 512, min((c + 1) * 512, V)
            if lo >= V:
                break
            vids = sigma[lo:hi]
            balanced[lo:hi] = _balance_chunk(vids, deg[vids], caps_c[c])
        sigma = balanced
        inv = np.empty(V, np.int64)
        inv[sigma] = np.arange(V)
        sigmas.append(sigma)
        invs.append(inv)
        corners_all.append(
            [[(pix[sigma], w[sigma]) for (pix, w) in _corners(grid, Wm)]
             for (_, Wm) in MAPS]
        )

    # sampling schedule: per (chunk, map) the union over meshes of touched
    # pixel tiles; ragged (no padding) ------------------------------------
    ntile_map = [(Wm * Wm + 127) // 128 for (_, Wm) in MAPS]
    g_off = np.cumsum([0] + ntile_map)  # global G-tile offsets
    sched = []  # sched[mi][c] = list of pixel-tile indices
    for mi in range(4):
        per_c = []
        for c in range(NVCH):
            lo, hi = c * 512, min((c + 1) * 512, V)
            tiles = set()
            if lo < V:
                for m in range(B):
                    for (pix, _w) in corners_all[m][mi]:
                        pc = pix[lo:hi] // 128
                        tiles.update(np.unique(pc).tolist())
            per_c.append(sorted(tiles) if tiles else [0])
        sched.append(per_c)
    npc = [sum(len(sched[mi][c]) for mi in range(4)) for c in range(NVCH)]
    npc_off = np.concatenate([[0], np.cumsum(npc)]).astype(int)
    npair = int(npc_off[-1])
    max_npc = max(npc)

    # graph structure ------------------------------------------------------
    # directed edges sorted by dst, grouped per dst tile; per-tile subchunk
    # count = max over meshes (keeps one SPMD instruction stream)
    ecnts = []
    esorted = []
    for m in range(B):
        e = edges[m * E_PER:(m + 1) * E_PER] - m * V
        a = invs[m][e[:, 0]]
        b = invs[m][e[:, 1]]
        dst = np.concatenate([a, b])
        src = np.concatenate([b, a])
        order = np.lexsort((src, dst))
        esorted.append((dst[order], src[order]))
        ecnts.append(np.bincount(dst // 128, minlength=NT))
    nsub_t = np.maximum(1, -(-np.stack(ecnts).max(axis=0) // 128))  # [NT]
    assert nsub_t.max() <= 8
    sub_off = np.concatenate([[0], np.cumsum(nsub_t)]).astype(int)
    tot_sub = int(sub_off[-1])
    sub_g_max = int(max(sub_off[(g + 1) * GT] - sub_off[g * GT]
                        for g in range(NGRP)))

    per_core = []
    for m in range(B):
        dst, src = esorted[m]
        counts = ecnts[m]
        src_slots = np.zeros((tot_sub, 128), np.int64)
        dl_slots = np.full((tot_sub, 128), -1, np.int32)
        pos = 0
        for t in range(NT):
            cnt = counts[t]
            so = sub_off[t] * 128
            src_slots.reshape(-1)[so:so + cnt] = src[pos:pos + cnt]
            dl_slots.reshape(-1)[so:so + cnt] = dst[pos:pos + cnt] - t * 128
            pos += cnt
        # remap src vertex id -> partition-major h1d row: (v%128)*NT + v//128
        src_lin = ((src_slots % 128) * NT + src_slots // 128).reshape(-1)
        # wrapped int16 for dma_gather: idx i at (i%16, i//16), replicated 8x
        srcw = np.tile(src_lin.reshape(-1, 16).T, (8, 1)).astype(np.int16)
        # dst_local per (partition, subchunk)
        dl = dl_slots.reshape(tot_sub, 128).T.copy().astype(ml_dtypes.bfloat16)

        # sampling blocks (ragged, chunk-major) ----------------------------
        wsc = np.zeros((npair, 128, 512), np.float32)
        pi = 0
        for c in range(NVCH):
            lo, hi = c * 512, min((c + 1) * 512, V)
            for mi in range(4):
                for t in sched[mi][c]:
                    blk = wsc[pi]
                    if lo < V:
                        for (pix, w) in corners_all[m][mi]:
                            px = pix[lo:hi]
                            sel = (px >= t * 128) & (px < (t + 1) * 128)
                            jj = np.nonzero(sel)[0]
                            np.add.at(blk, (px[jj] - t * 128, jj), w[lo:hi][jj])
                    pi += 1
        assert pi == npair

        vt = np.zeros((3, VP), np.float32)
        vt[:, :V] = verts[m * V:(m + 1) * V][sigmas[m]].T

        aux = {
            "f1": np.ascontiguousarray(feats[0][m].reshape(256, -1)).astype(BF),
            "f2": np.ascontiguousarray(feats[1][m].reshape(512, -1)).astype(BF),
            "f3": np.ascontiguousarray(feats[2][m].reshape(1024, -1)).astype(BF),
            "f4": np.ascontiguousarray(feats[3][m].reshape(2048, -1)).astype(BF),
            "bw": np.ascontiguousarray(
                np.asarray(inputs["bottleneck_w"], np.float32)
                .reshape(30, 128, HID).transpose(1, 0, 2)
                .reshape(128, 30 * HID)).astype(BF),
            "wsc": wsc.reshape(npair * 128, 512).astype(F8),
            "srcw": np.ascontiguousarray(srcw),
            "dstloc": np.ascontiguousarray(dl),
            "iota": np.tile(np.arange(128, dtype=BF), (128, 1)),
            "vertsT": vt.astype(BF),
            "encc": enc[m].reshape(2, 128).T.copy().astype(BF),  # [128, 2]
            "g0w0m": np.asarray(inputs["g0_w0"][:128], np.float32).astype(BF),
            "g0w0v": np.asarray(inputs["g0_w0"][128:131], np.float32).astype(BF),
            "g0w0e": np.ascontiguousarray(
                np.asarray(inputs["g0_w0"][131:387], np.float32)).astype(BF),
            "g0w1m": np.asarray(inputs["g0_w1"][:128], np.float32).astype(BF),
            "g0w1v": np.asarray(inputs["g0_w1"][128:131], np.float32).astype(BF),
            "g0w1e": np.ascontiguousarray(
                np.asarray(inputs["g0_w1"][131:387], np.float32)).astype(BF),
            "gw0": np.ascontiguousarray(
                np.asarray(inputs["gw0"], np.float32).transpose(1, 0, 2)
                .reshape(128, 7 * 128)).astype(BF),
            "gw1": np.ascontiguousarray(
                np.asarray(inputs["gw1"], np.float32).transpose(1, 0, 2)
                .reshape(128, 7 * 128)).astype(BF),
            "offw": np.asarray(inputs["off_w"], np.float32).astype(BF),
        }
        per_core.append(aux)

    cfg = {"sched": sched, "npc": npc, "npc_off": npc_off.tolist(),
           "npair": npair, "max_npc": max_npc,
           "g_off": g_off.tolist(), "ntile_map": ntile_map,
           "nsub_t": nsub_t.tolist(), "sub_off": sub_off.tolist(),
           "tot_sub": tot_sub, "sub_g_max": sub_g_max}
    post = {"sigmas": sigmas}
    return cfg, per_core, post


def _build(cfg, shapes, dump=None, nlayers=8, repeat=1):
    """Build the SPMD Bass program (same instruction stream for all cores)."""
    nc = bacc.Bacc("TRN2", target_bir_lowering=False, debug=False, num_devices=B)
    ap = {}
    for name, arr in shapes.items():
        ap[name] = nc.dram_tensor(
            name, list(arr.shape), mybir.dt.from_np(arr.dtype),
            kind="ExternalInput").ap()
    out = nc.dram_tensor("out", [3, VP], F32, kind="ExternalOutput").ap()
    xdump = (nc.dram_tensor("xdump", [128, VP], F32, kind="ExternalOutput").ap()
             if dump else None)
    h1d2 = [nc.dram_tensor("h1da", [VP, HID], BF16).ap(),
            nc.dram_tensor("h1db", [VP, HID], BF16).ap()]

    sched = cfg["sched"]
    npc = cfg["npc"]
    npc_off = cfg["npc_off"]
    max_npc = cfg["max_npc"]
    g_off = cfg["g_off"]
    ntile_map = cfg["ntile_map"]
    NGT = g_off[4]  # total G tiles
    tot_sub = cfg["tot_sub"]
    nsub_t = cfg["nsub_t"]
    sub_off = cfg["sub_off"]
    sub_g_max = cfg["sub_g_max"]

    with tile.TileContext(nc) as tc, ExitStack() as ctx:
        # ---------------- persistent pools ----------------
        s_pers = sub_off[G_PERS * GT]  # persistent one-hot subchunks
        pp = ctx.enter_context(tc.tile_pool(name="pers", bufs=1))
        xa = pp.tile([128, VP], BF16, tag="xa")
        xb = pp.tile([128, VP], BF16, tag="xb")
        oh_pers = pp.tile([128, s_pers, 128], BF16, tag="ohp")
        srcw_t = pp.tile([128, tot_sub * 8], I16, tag="srcw")
        dstloc_t = pp.tile([128, tot_sub, 1], BF16, tag="dstloc")
        iota_t = pp.tile([128, 1, 128], BF16, tag="iota")
        w0_t = pp.tile([128, 7 * 128], BF16, tag="w0")
        w1_t = pp.tile([128, 7 * 128], BF16, tag="w1")
        g0_t = pp.tile([128, 6 * 128], BF16, tag="g0")  # w0m,w1m,w0e(2),w1e(2)
        g0v_t = pp.tile([3, 256], BF16, tag="g0v")      # w0v, w1v
        offw_t = pp.tile([128, 3], BF16, tag="offw")
        ones_t = pp.tile([1, GT * 128], BF16, tag="ones")
        erow_t = pp.tile([1, 256], BF16, tag="erow")    # e0row, e1row
        encc_t = pp.tile([128, 2], BF16, tag="encc")

        nc.vector.memset(ones_t[:], 1.0)

        psA = ctx.enter_context(tc.tile_pool(name="psA", bufs=2, space="PSUM"))

        def _load_g0():
            """Layer-0 weight loads + enc rank-1 rows; issued after the first
            feature-map DMAs so they don't delay the sampling pipeline."""
            nc.sync.dma_start(g0_t[:, 0:128], ap["g0w0m"][:])
            nc.sync.dma_start(g0_t[:, 128:256], ap["g0w1m"][:])
            nc.sync.dma_start(
                g0_t[:, 256:512].rearrange("p (c h) -> p c h", h=128),
                ap["g0w0e"].rearrange("(c p) h -> p c h", p=128))
            nc.sync.dma_start(
                g0_t[:, 512:768].rearrange("p (c h) -> p c h", h=128),
                ap["g0w1e"].rearrange("(c p) h -> p c h", p=128))
            nc.sync.dma_start(g0v_t[:, 0:128], ap["g0w0v"][:])
            nc.sync.dma_start(g0v_t[:, 128:256], ap["g0w1v"][:])
            nc.sync.dma_start(offw_t[:], ap["offw"][:])
            nc.sync.dma_start(encc_t[:], ap["encc"][:])
            _load_bulk()
            # enc rank-1 rows: e{0,1} = g0_w{0,1}[131:387].T @ enc -> [1,128]
            for k in range(2):
                pe = psA.tile([1, 128], F32, tag="p1")
                for cchunk in range(2):
                    nc.tensor.matmul(
                        out=pe[:],
                        lhsT=encc_t[:, cchunk:cchunk + 1],
                        rhs=g0_t[:, 256 + k * 256 + cchunk * 128:
                                 256 + k * 256 + cchunk * 128 + 128],
                        start=(cchunk == 0), stop=(cchunk == 1))
                nc.scalar.activation(erow_t[:, k * 128:(k + 1) * 128], pe[:],
                                     AF.Copy)

        def _load_bulk():
            # bulky graph-structure loads on the Activation queue: no waits,
            # so they drain mid-sampling and fill idle DMA bandwidth without
            # head-of-line blocking the SP wsc prefetch stream
            nc.scalar.dma_start(srcw_t[:], ap["srcw"][:])
            nc.scalar.dma_start(
                dstloc_t[:], ap["dstloc"].rearrange("p (s o) -> p s o", o=1))
            nc.scalar.dma_start(iota_t[:].rearrange("p o d -> p (o d)"),
                                ap["iota"][:])
            nc.scalar.dma_start(w0_t[:], ap["gw0"][:])
            nc.scalar.dma_start(w1_t[:], ap["gw1"][:])

        def _sampling(sctx):
            """Phase 1: vert_align sampling -> xa (bf16 columns).  The layer-0
            h1 rows are produced chunk-by-chunk right after each ReLU so the
            first gathers can start as soon as sampling ends."""
            sp = sctx.enter_context(tc.tile_pool(name="samp", bufs=1))
            spfm = sctx.enter_context(tc.tile_pool(name="sampfm", bufs=2))
            spf = sctx.enter_context(tc.tile_pool(name="sampf", bufs=2))
            spw = sctx.enter_context(tc.tile_pool(name="sampw", bufs=4))
            sph = sctx.enter_context(tc.tile_pool(name="samph", bufs=2))
            spp1 = sctx.enter_context(tc.tile_pool(name="samppsum1", bufs=2,
                                                   space="PSUM"))
            spp2 = sctx.enter_context(tc.tile_pool(name="samppsum2", bufs=2,
                                                   space="PSUM"))
            g_sb = sp.tile([128, NGT * 128], BF16, tag="gsb")

            def _load_map(mi):
                C, Wm = MAPS[mi]
                ncc = C // 128
                bw_t = spf.tile([128, 16 * 128], BF16, tag="bw")
                nc.sync.dma_start(
                    bw_t[:, :ncc * 128],
                    ap["bw"][:, CH_OFF[mi]:CH_OFF[mi] + ncc * 128])
                fm_t = spfm.tile([128, 2 * 3136], BF16, tag="fm")
                nc.sync.dma_start(
                    fm_t[:, :ncc * Wm * Wm].rearrange(
                        "p (c hw) -> p c hw", c=ncc),
                    ap[f"f{mi+1}"].rearrange("(c p) hw -> p c hw", p=128))
                return fm_t, bw_t

            nxt_ld = _load_map(0)
            _load_g0()
            for mi, (C, Wm) in enumerate(MAPS):
                HW = Wm * Wm
                ncc = C // 128
                fm_t, bw_t = nxt_ld
                if mi + 1 < 4:
                    nxt_ld = _load_map(mi + 1)
                for t in range(ntile_map[mi]):
                    p0 = t * 128
                    pcnt = min(128, HW - p0)
                    pg = psA.tile([128, 128], F32, tag="p1")
                    for cc in range(ncc):
                        nc.tensor.matmul(
                            out=pg[:pcnt, :],
                            lhsT=fm_t[:, cc * HW + p0:cc * HW + p0 + pcnt],
                            rhs=bw_t[:, cc * 128:cc * 128 + 128],
                            start=(cc == 0), stop=(cc == ncc - 1))
                    gt = g_off[mi] + t
                    nc.scalar.activation(
                        g_sb[:pcnt, gt * 128:gt * 128 + 128], pg[:pcnt, :],
                        AF.Copy)

            for c in range(NVCH):
                ps = spp1.tile([128, 512], F32, tag="ps")
                pairs_c = []
                for mi in range(4):
                    for t in sched[mi][c]:
                        pairs_c.append((mi, t))
                assert len(pairs_c) == npc[c]
                half = (max_npc + 1) // 2
                nh = (npc[c] + half - 1) // half
                wts = []
                for hb in range(nh):
                    k0, k1 = hb * half, min((hb + 1) * half, npc[c])
                    wt = spw.tile([128, half, 512], FP8, tag="wsc")
                    nc.sync.dma_start(
                        wt[:, :k1 - k0, :],
                        ap["wsc"].rearrange("(k p) h -> p k h", p=128)
                        [:, npc_off[c] + k0:npc_off[c] + k1, :])
                    wts.append(wt)
                for k, (mi, t) in enumerate(pairs_c):
                    HW = MAPS[mi][1] ** 2
                    pcnt = min(128, HW - t * 128)
                    gt = g_off[mi] + t
                    nc.tensor.matmul(
                        out=ps[:],
                        lhsT=g_sb[:pcnt, gt * 128:gt * 128 + 128],
                        rhs=wts[k // half][:pcnt, k % half, :],
                        start=(k == 0), stop=(k == len(pairs_c) - 1))
                nc.scalar.activation(xa[:, c * 512:(c + 1) * 512], ps[:],
                                     AF.Relu)
                # layer-0 h1 rows for this chunk's 4 tiles
                vv = sph.tile([3, 512], BF16, tag="vt")
                nc.sync.dma_start(vv[:],
                                  ap["vertsT"][:, c * 512:(c + 1) * 512])
                ph4 = spp2.tile([128, 512], F32, tag="ph4")
                hstc = sph.tile([128, 512], BF16, tag="hstc")
                for ti in range(4):
                    t = 4 * c + ti
                    sl = slice(ti * 128, (ti + 1) * 128)
                    nc.tensor.matmul(
                        out=ph4[:, sl], lhsT=xa[:, t * 128:(t + 1) * 128],
                        rhs=g0_t[:, 128:256], start=True, stop=False)
                    nc.tensor.matmul(
                        out=ph4[:, sl], lhsT=vv[:, sl],
                        rhs=g0v_t[:, 128:256], start=False, stop=False)
                    nc.tensor.matmul(
                        out=ph4[:, sl], lhsT=ones_t[:, 0:128],
                        rhs=erow_t[:, 128:256], start=False, stop=True)
                nc.scalar.activation(hstc[:], ph4[:], AF.Copy)
                # Pool-issued so a write waiting on compute never head-of-line
                # blocks the SP queue's wsc prefetch stream.
                h1_writes.append(nc.gpsimd.dma_start(
                    h1d2[0].rearrange("(p n) c -> p n c", p=128)
                    [:, c * 4:(c + 1) * 4, :],
                    hstc[:].rearrange("p (n c) -> p n c", c=128)))

        h1_writes = []
        with ExitStack() as sctx:
            _sampling(sctx)


        # ---------------- phase 2: graph conv layers ----------------
        lp = ctx.enter_context(tc.tile_pool(name="lay", bufs=3))
        lpo = ctx.enter_context(tc.tile_pool(name="layoh", bufs=2))
        lph = ctx.enter_context(tc.tile_pool(name="layh", bufs=2))
        lpv = ctx.enter_context(tc.tile_pool(name="layv", bufs=2))
        psx = ctx.enter_context(tc.tile_pool(name="psumx", bufs=2, space="PSUM"))
        psB = ctx.enter_context(tc.tile_pool(name="psumo", bufs=1, space="PSUM"))

        def _layers(first_rep, last_rep, h1_writes):
            cur, nxt = xa, xb
            if not first_rep:
                # prologue: recompute layer-0 h1 rows (repeat mode only)
                h1_writes = []
                for g in range(NGRP):
                    hst = lph.tile([128, GT * 128], BF16, tag="hstg")
                    vv = lpv.tile([3, GT * 128], BF16, tag="vt")
                    nc.sync.dma_start(
                        vv[:], ap["vertsT"][:, g * 512:(g + 1) * 512])
                    ph4 = psx.tile([128, 512], F32, tag="ph4")
                    for ti in range(GT):
                        t = g * GT + ti
                        sl = slice(ti * 128, (ti + 1) * 128)
                        nc.tensor.matmul(
                            out=ph4[:, sl], lhsT=cur[:, t * 128:(t + 1) * 128],
                            rhs=g0_t[:, 128:256], start=True, stop=False)
                        nc.tensor.matmul(
                            out=ph4[:, sl], lhsT=vv[:, sl],
                            rhs=g0v_t[:, 128:256], start=False, stop=False)
                        nc.tensor.matmul(
                            out=ph4[:, sl], lhsT=ones_t[:, 0:128],
                            rhs=erow_t[:, 128:256], start=False, stop=True)
                    nc.scalar.activation(hst[:], ph4[:], AF.Copy)
                    h1_writes.append(nc.sync.dma_start(
                        h1d2[0].rearrange("(p n) c -> p n c", p=128)
                        [:, g * GT:(g + 1) * GT, :],
                        hst[:].rearrange("p (n c) -> p n c", c=128)))

            for l in range(nlayers):
                h1d = h1d2[l % 2]
                h1d_nxt = h1d2[(l + 1) % 2]
                next_writes = []

                # gather groups + scatter matmuls; h1 rows for layer l+1 are
                # produced group-by-group right after each ReLU so the next
                # layer's gathers can start almost immediately.
                for g in range(NGRP):
                    s0 = sub_off[g * GT]
                    s1 = sub_off[min((g + 1) * GT, NT)]
                    ng = s1 - s0
                    msg = lp.tile([128, sub_g_max, 128], BF16, tag="msg")
                    gi = nc.gpsimd.dma_gather(
                        out_ap=msg[:, :ng, :],
                        in_ap=h1d[:],
                        idxs_ap=srcw_t[:, s0 * 8:s1 * 8],
                        num_idxs=ng * 128,
                        num_idxs_reg=ng * 128,
                        elem_size=HID,
                        single_packet=False,
                    )
                    for wi in h1_writes:
                        tile.add_dep_helper(gi.ins, wi.ins,
                                            reason="h1 RAW: gather after write")
                    if g < G_PERS:
                        oh_t, so = oh_pers, 0
                        if l == 0 and first_rep:
                            # build the persistent one-hots (layer-invariant)
                            nc.vector.tensor_tensor(
                                out=oh_pers[:, s0:s1, :],
                                in0=dstloc_t[:, s0:s1, :]
                                .to_broadcast([128, ng, 128]),
                                in1=iota_t[:].to_broadcast([128, ng, 128]),
                                op=mybir.AluOpType.is_equal)
                    else:
                        oh_t = lpo.tile([128, sub_g_max, 128], BF16, tag="oht")
                        so = s0
                        nc.vector.tensor_tensor(
                            out=oh_t[:, :ng, :],
                            in0=dstloc_t[:, s0:s1, :]
                            .to_broadcast([128, ng, 128]),
                            in1=iota_t[:].to_broadcast([128, ng, 128]),
                            op=mybir.AluOpType.is_equal)
                    if l == 0:
                        vv2 = lpv.tile([3, GT * 128], BF16, tag="vt2")
                        nc.sync.dma_start(
                            vv2[:],
                            ap["vertsT"][:, g * GT * 128:(g + 1) * GT * 128])
                    W = GT * 128
                    px = psx.tile([128, W], F32, tag="px")
                    if l == 0:
                        nc.tensor.matmul(
                            out=px[:], lhsT=g0_t[:, 0:128],
                            rhs=cur[:, g * W:(g + 1) * W],
                            start=True, stop=False)
                        nc.tensor.matmul(
                            out=px[:], lhsT=g0v_t[:, 0:128],
                            rhs=vv2[:], start=False, stop=False)
                        nc.tensor.matmul(
                            out=px[:], lhsT=erow_t[:, 0:128],
                            rhs=ones_t[:], start=False, stop=False)
                    else:
                        nc.tensor.matmul(
                            out=px[:], lhsT=w0_t[:, (l - 1) * 128:l * 128],
                            rhs=cur[:, g * W:(g + 1) * W],
                            start=True, stop=False)
                    for ti in range(GT):
                        t = g * GT + ti
                        nst = nsub_t[t]
                        for j in range(nst):
                            s = sub_off[t] - s0 + j
                            nc.tensor.matmul(
                                out=px[:, ti * 128:(ti + 1) * 128],
                                lhsT=msg[:, s, :],
                                rhs=oh_t[:, sub_off[t] + j - so, :],
                                start=False,
                                stop=(ti == GT - 1 and j == nst - 1),
                                skip_group_check=True)
                    nc.scalar.activation(nxt[:, g * W:(g + 1) * W], px[:],
                                         AF.Relu)
                    if l == nlayers - 1 and last_rep:
                        # delta_v for this group: off_w.T @ x cols -> [3, 512]
                        po = psB.tile([3, GT * 128], F32, tag="po")
                        nc.tensor.matmul(
                            out=po[:], lhsT=offw_t[:],
                            rhs=nxt[:, g * W:(g + 1) * W],
                            start=True, stop=True)
                        ost = lph.tile([3, GT * 128], F32, tag="ost")
                        nc.scalar.activation(ost[:], po[:], AF.Copy)
                        nc.sync.dma_start(out[:, g * W:(g + 1) * W], ost[:])
                    if l + 1 < nlayers:
                        # h1 rows for layer l+1 on this group's tiles
                        hst = lph.tile([128, GT * 128], BF16, tag="hstg")
                        ph4 = psx.tile([128, 512], F32, tag="ph4")
                        for ti in range(GT):
                            t = g * GT + ti
                            nc.tensor.matmul(
                                out=ph4[:, ti * 128:(ti + 1) * 128],
                                lhsT=nxt[:, t * 128:(t + 1) * 128],
                                rhs=w1_t[:, l * 128:(l + 1) * 128],
                                start=True, stop=True)
                        nc.scalar.activation(hst[:], ph4[:], AF.Copy)
                        next_writes.append(nc.sync.dma_start(
                            h1d_nxt.rearrange("(p n) c -> p n c", p=128)
                            [:, g * GT:(g + 1) * GT, :],
                            hst[:].rearrange("p (n c) -> p n c", c=128)))
                h1_writes = next_writes
                cur, nxt = nxt, cur

        for _rep in range(repeat):
            _layers(_rep == 0, _rep == repeat - 1, h1_writes)
        cur = xa if nlayers % 2 == 0 else xb

        if xdump is not None:
            nc.sync.dma_start(xdump[:], cur[:])

        if nlayers == 0:
            # output straight from the sampled activations (debug path)
            for g in range(NGRP):
                po = psB.tile([3, GT * 128], F32, tag="po")
                nc.tensor.matmul(
                    out=po[:], lhsT=offw_t[:],
                    rhs=cur[:, g * 512:(g + 1) * 512], start=True, stop=True)
                ost = lph.tile([3, GT * 128], F32, tag="ost")
                nc.scalar.activation(ost[:], po[:], AF.Copy)
                nc.sync.dma_start(out[:, g * 512:(g + 1) * 512], ost[:])

    nc.compile()
    return nc


_CACHE = {}


def kernel(**inputs) -> np.ndarray:
    cfg, per_core, post = _prep(inputs)
    key = (cfg["npair"], tuple(cfg["npc"]), cfg["tot_sub"],
           tuple(cfg["nsub_t"]))
    if key not in _CACHE:
        _CACHE[key] = _build(cfg, per_core[0])
    nc = _CACHE[key]
    res = run_bass_kernel_spmd(nc, per_core, list(range(B)))
    outs = np.empty((B, V, 3), np.float32)
    for m in range(B):
        rows = np.ascontiguousarray(res.results[m]["out"].T)[:V]
        outs[m][post["sigmas"][m]] = rows
    return outs.reshape(B * V, 3)


if __name__ == "__main__":
    pass


# revision 61
# speedup vs baseline: 1.0221x; 1.0095x over previous
"""Trainium2 Bass kernel for DeformationNetworkGraphConvolutionalFullRes.

Full (unsharded) inputs in, full output out. Data-parallel over the 4 meshes:
core m processes mesh m (cores 4-7 idle). Inside each core:

  - vert_align sampling is computed as (S @ F) @ W == S @ (F @ W): per feature
    map, F[C,HW] @ Wslice[C,128] -> G[HW,128] (tiny matmuls), then the sparse
    bilinear operator S (4 nonzeros/row) is applied as dense [128px, 512vert]
    blocks (built host-side from the vertex coordinates) streamed into the
    TensorEngine, accumulating over maps/pixel-tiles in PSUM. Vertices are
    pre-sorted by image cell so each 512-vertex chunk touches few pixel tiles.
  - Each GraphConv layer: h1 = x@W1 rows are written to HBM in a
    partition-major layout (full-bandwidth writes); messages h1[src] are
    pulled with dma_gather in dst-sorted edge order; the segmented sum over
    edges is done as one-hot matmuls accumulating in PSUM on top of
    h0 = x@W0 (+ rank-1 image-encoding term), then ReLU writes the
    transposed activations for the next layer directly.

Optimizations vs the original version:
  - everything streams in bf16 (activations, weights, feature maps, sampling
    blocks); PSUM accumulation stays fp32.
  - per-mesh degree-balanced vertex->tile packing (within each 512-vertex
    sampling chunk) minimizes edge-subchunk padding across the SPMD cores.
  - h1 rows live in DRAM partition-major so the per-layer write runs at full
    DMA bandwidth.
  - the scatter one-hots are built once (layer 0) and persist in SBUF.
  - the sampling schedule is ragged (no zero-block padding).
"""

import ml_dtypes
import numpy as np
from contextlib import ExitStack

import concourse.bass as bass
import concourse.tile as tile
from concourse import bacc, mybir
from concourse.bass_utils import run_bass_kernel_spmd

# ---------------- problem constants (hardcoded per spec) ----------------
B = 4
V = 10242
E_PER = 30720
HID = 128
MAPS = [(256, 56), (512, 28), (1024, 14), (2048, 7)]  # (C, H==W)
CH_OFF = [0, 256, 768, 1792, 3840]

VP = 10752            # padded vertex count: 21 chunks of 512 = 84 tiles of 128
NT = VP // 128        # 84 vertex tiles
NVCH = VP // 512      # 21 vertex chunks (sampling)
GT = 4                # dst tiles per gather group (group == sampling chunk)
NGRP = NT // GT       # 21 gather groups
G_PERS = 14           # groups with persistent one-hots (rest rebuilt per layer)
HB = 6                # h1 write batch (tiles, layer-0 prologue)

F32 = mybir.dt.float32
BF16 = mybir.dt.bfloat16
FP8 = mybir.dt.float8e4
I32 = mybir.dt.int32
I16 = mybir.dt.int16
AF = mybir.ActivationFunctionType
BF = ml_dtypes.bfloat16
F8 = ml_dtypes.float8_e4m3fn


def _corners(grid, W):
    """grid [V,2] in [-1,1] -> list of (pix_idx int32, weight f32) per corner."""
    x = (grid[:, 0] + 1.0) * 0.5 * (W - 1)
    y = (grid[:, 1] + 1.0) * 0.5 * (W - 1)
    x0f, y0f = np.floor(x), np.floor(y)
    wx1, wy1 = (x - x0f).astype(np.float32), (y - y0f).astype(np.float32)
    wx0, wy0 = 1.0 - wx1, 1.0 - wy1
    x0 = np.clip(x0f, 0, W - 1).astype(np.int64)
    x1 = np.clip(x0f + 1, 0, W - 1).astype(np.int64)
    y0 = np.clip(y0f, 0, W - 1).astype(np.int64)
    y1 = np.clip(y0f + 1, 0, W - 1).astype(np.int64)
    return [
        (y0 * W + x0, wy0 * wx0),
        (y0 * W + x1, wy0 * wx1),
        (y1 * W + x0, wy1 * wx0),
        (y1 * W + x1, wy1 * wx1),
    ]


def _balance_chunk(vids, degs, caps):
    """Assign the chunk's vertices to 4 tiles of 128 slots, packing each tile
    to at most caps[t] edge load where possible (caps are multiples of 128).
    Returns the vertex ids in new order (tile-by-tile)."""
    order = np.argsort(-degs, kind="stable")
    slots = [0, 0, 0, 0]
    loads = [0.0, 0.0, 0.0, 0.0]
    buckets = [[], [], [], []]
    for i in order:
        d = degs[i]
        # most-headroom tile that still fits under its cap
        best, best_room = -1, -1.0
        for t in range(4):
            room = caps[t] - loads[t]
            if slots[t] < 128 and room >= d and room > best_room:
                best, best_room = t, room
        if best < 0:  # cap bust: most headroom among tiles with free slots
            cands = [t for t in range(4) if slots[t] < 128]
            best = max(cands, key=lambda t: caps[t] - loads[t])
        buckets[best].append(vids[i])
        slots[best] += 1
        loads[best] += d
    out = []
    for t in range(4):
        out.extend(buckets[t])
    return np.array(out, dtype=np.int64)


def _prep(inputs):
    """Host-side restructuring: sorting, balancing, padding, index tables,
    sparse-operator blocks. Returns (cfg, per_core_aux_list, post)."""
    feats = [inputs["feat1"], inputs["feat2"], inputs["feat3"], inputs["feat4"]]
    av = np.asarray(inputs["aligned_verts"], np.float32)
    verts = np.asarray(inputs["verts_packed"], np.float32)
    enc = np.asarray(inputs["image_enc"], np.float32)
    edges = np.asarray(inputs["edges"], np.int64)

    for bn in ["bottleneck_b", "g0_b0", "g0_b1", "off_b"]:
        assert not np.any(np.asarray(inputs[bn])), f"{bn} nonzero: unsupported"
    assert not np.any(np.asarray(inputs["gb0"])) and not np.any(
        np.asarray(inputs["gb1"])
    ), "gb nonzero: unsupported"

    # per-mesh vertex sort (by finest-map cell) + degree balance ------------
    # chunk membership is fixed by the cell sort; only the tile assignment
    # within each 512-vertex chunk is rebalanced.  The per-chunk subchunk
    # budget K_c comes from the max-over-meshes chunk load, spread evenly
    # over the 4 tiles (shared SPMD structure).
    sigmas0, degs_all = [], []
    for m in range(B):
        grid = av[m, :, :2]
        cs = _corners(grid, MAPS[0][1])
        key = cs[0][0]  # y0*56+x0 of map 0
        sigmas0.append(np.argsort(key, kind="stable"))
        e = edges[m * E_PER:(m + 1) * E_PER] - m * V
        degs_all.append(np.bincount(np.concatenate([e[:, 0], e[:, 1]]),
                                    minlength=V).astype(np.float64))
    caps_c = []
    for c in range(NVCH):
        lo, hi = c * 512, min((c + 1) * 512, V)
        tmax = 0.0
        if lo < V:
            for m in range(B):
                tmax = max(tmax, degs_all[m][sigmas0[m][lo:hi]].sum())
        kc = max(4, int(-(-tmax // 128)))
        base, rem = kc // 4, kc % 4
        caps_c.append([128 * (base + (1 if i < rem else 0)) for i in range(4)])

    sigmas, invs, corners_all = [], [], []
    for m in range(B):
        grid = av[m, :, :2]
        sigma = sigmas0[m]
        deg = degs_all[m]
        balanced = np.empty_like(sigma)
        for c in range(NVCH):
            lo, hi = c * 512, min((c + 1) * 512, V)
            if lo >= V:
                break
            vids = sigma[lo:hi]
            balanced[lo:hi] = _balance_chunk(vids, deg[vids], caps_c[c])
        sigma = balanced
        inv = np.empty(V, np.int64)
        inv[sigma] = np.arange(V)
        sigmas.append(sigma)
        invs.append(inv)
        corners_all.append(
            [[(pix[sigma], w[sigma]) for (pix, w) in _corners(grid, Wm)]
             for (_, Wm) in MAPS]
        )

    # sampling schedule: per (chunk, map) the union over meshes of touched
    # pixel tiles; ragged (no padding) ------------------------------------
    ntile_map = [(Wm * Wm + 127) // 128 for (_, Wm) in MAPS]
    g_off = np.cumsum([0] + ntile_map)  # global G-tile offsets
    sched = []  # sched[mi][c] = list of pixel-tile indices
    for mi in range(4):
        per_c = []
        for c in range(NVCH):
            lo, hi = c * 512, min((c + 1) * 512, V)
            tiles = set()
            if lo < V:
                for m in range(B):
                    for (pix, _w) in corners_all[m][mi]:
                        pc = pix[lo:hi] // 128
                        tiles.update(np.unique(pc).tolist())
            per_c.append(sorted(tiles) if tiles else [0])
        sched.append(per_c)
    npc = [sum(len(sched[mi][c]) for mi in range(4)) for c in range(NVCH)]
    npc_off = np.concatenate([[0], np.cumsum(npc)]).astype(int)
    npair = int(npc_off[-1])
    max_npc = max(npc)

    # graph structure ------------------------------------------------------
    # directed edges sorted by dst, grouped per dst tile; per-tile subchunk
    # count = max over meshes (keeps one SPMD instruction stream)
    ecnts = []
    esorted = []
    for m in range(B):
        e = edges[m * E_PER:(m + 1) * E_PER] - m * V
        a = invs[m][e[:, 0]]
        b = invs[m][e[:, 1]]
        dst = np.concatenate([a, b])
        src = np.concatenate([b, a])
        order = np.lexsort((src, dst))
        esorted.append((dst[order], src[order]))
        ecnts.append(np.bincount(dst // 128, minlength=NT))
    # 0 subchunks for tiles with no edges in any mesh (the pad tiles)
    nsub_t = -(-np.stack(ecnts).max(axis=0) // 128)  # [NT]
    assert nsub_t.max() <= 8
    sub_off = np.concatenate([[0], np.cumsum(nsub_t)]).astype(int)
    tot_sub = int(sub_off[-1])
    sub_g_max = int(max(sub_off[(g + 1) * GT] - sub_off[g * GT]
                        for g in range(NGRP)))

    per_core = []
    for m in range(B):
        dst, src = esorted[m]
        counts = ecnts[m]
        src_slots = np.zeros((tot_sub, 128), np.int64)
        dl_slots = np.full((tot_sub, 128), -1, np.int32)
        pos = 0
        for t in range(NT):
            cnt = counts[t]
            so = sub_off[t] * 128
            src_slots.reshape(-1)[so:so + cnt] = src[pos:pos + cnt]
            dl_slots.reshape(-1)[so:so + cnt] = dst[pos:pos + cnt] - t * 128
            pos += cnt
        # remap src vertex id -> partition-major h1d row: (v%128)*NT + v//128
        src_lin = ((src_slots % 128) * NT + src_slots // 128).reshape(-1)
        # wrapped int16 for dma_gather: idx i at (i%16, i//16), replicated 8x
        srcw = np.tile(src_lin.reshape(-1, 16).T, (8, 1)).astype(np.int16)
        # dst_local per (partition, subchunk)
        dl = dl_slots.reshape(tot_sub, 128).T.copy().astype(ml_dtypes.bfloat16)

        # sampling blocks (ragged, chunk-major) ----------------------------
        wsc = np.zeros((npair, 128, 512), np.float32)
        pi = 0
        for c in range(NVCH):
            lo, hi = c * 512, min((c + 1) * 512, V)
            for mi in range(4):
                for t in sched[mi][c]:
                    blk = wsc[pi]
                    if lo < V:
                        for (pix, w) in corners_all[m][mi]:
                            px = pix[lo:hi]
                            sel = (px >= t * 128) & (px < (t + 1) * 128)
                            jj = np.nonzero(sel)[0]
                            np.add.at(blk, (px[jj] - t * 128, jj), w[lo:hi][jj])
                    pi += 1
        assert pi == npair

        vt = np.zeros((3, VP), np.float32)
        vt[:, :V] = verts[m * V:(m + 1) * V][sigmas[m]].T

        aux = {
            "f1": np.ascontiguousarray(feats[0][m].reshape(256, -1)).astype(BF),
            "f2": np.ascontiguousarray(feats[1][m].reshape(512, -1)).astype(BF),
            "f3": np.ascontiguousarray(feats[2][m].reshape(1024, -1)).astype(BF),
            "f4": np.ascontiguousarray(feats[3][m].reshape(2048, -1)).astype(BF),
            "bw": np.ascontiguousarray(
                np.asarray(inputs["bottleneck_w"], np.float32)
                .reshape(30, 128, HID).transpose(1, 0, 2)
                .reshape(128, 30 * HID)).astype(BF),
            "wsc": wsc.reshape(npair * 128, 512).astype(F8),
            "srcw": np.ascontiguousarray(srcw),
            "dstloc": np.ascontiguousarray(dl),
            "iota": np.tile(np.arange(128, dtype=BF), (128, 1)),
            "vertsT": vt.astype(BF),
            "encc": enc[m].reshape(2, 128).T.copy().astype(BF),  # [128, 2]
            "g0w0m": np.asarray(inputs["g0_w0"][:128], np.float32).astype(BF),
            "g0w0v": np.asarray(inputs["g0_w0"][128:131], np.float32).astype(BF),
            "g0w0e": np.ascontiguousarray(
                np.asarray(inputs["g0_w0"][131:387], np.float32)).astype(BF),
            "g0w1m": np.asarray(inputs["g0_w1"][:128], np.float32).astype(BF),
            "g0w1v": np.asarray(inputs["g0_w1"][128:131], np.float32).astype(BF),
            "g0w1e": np.ascontiguousarray(
                np.asarray(inputs["g0_w1"][131:387], np.float32)).astype(BF),
            "gw0": np.ascontiguousarray(
                np.asarray(inputs["gw0"], np.float32).transpose(1, 0, 2)
                .reshape(128, 7 * 128)).astype(BF),
            "gw1": np.ascontiguousarray(
                np.asarray(inputs["gw1"], np.float32).transpose(1, 0, 2)
                .reshape(128, 7 * 128)).astype(BF),
            "offw": np.asarray(inputs["off_w"], np.float32).astype(BF),
        }
        per_core.append(aux)

    cfg = {"sched": sched, "npc": npc, "npc_off": npc_off.tolist(),
           "npair": npair, "max_npc": max_npc,
           "g_off": g_off.tolist(), "ntile_map": ntile_map,
           "nsub_t": nsub_t.tolist(), "sub_off": sub_off.tolist(),
           "tot_sub": tot_sub, "sub_g_max": sub_g_max}
    post = {"sigmas": sigmas}
    return cfg, per_core, post


def _build(cfg, shapes, dump=None, nlayers=8, repeat=1):
    """Build the SPMD Bass program (same instruction stream for all cores)."""
    nc = bacc.Bacc("TRN2", target_bir_lowering=False, debug=False, num_devices=B)
    ap = {}
    for name, arr in shapes.items():
        ap[name] = nc.dram_tensor(
            name, list(arr.shape), mybir.dt.from_np(arr.dtype),
            kind="ExternalInput").ap()
    out = nc.dram_tensor("out", [3, VP], F32, kind="ExternalOutput").ap()
    xdump = (nc.dram_tensor("xdump", [128, VP], F32, kind="ExternalOutput").ap()
             if dump else None)
    h1d2 = [nc.dram_tensor("h1da", [VP, HID], BF16).ap(),
            nc.dram_tensor("h1db", [VP, HID], BF16).ap()]

    sched = cfg["sched"]
    npc = cfg["npc"]
    npc_off = cfg["npc_off"]
    max_npc = cfg["max_npc"]
    g_off = cfg["g_off"]
    ntile_map = cfg["ntile_map"]
    NGT = g_off[4]  # total G tiles
    tot_sub = cfg["tot_sub"]
    nsub_t = cfg["nsub_t"]
    sub_off = cfg["sub_off"]
    sub_g_max = cfg["sub_g_max"]

    with tile.TileContext(nc) as tc, ExitStack() as ctx:
        # ---------------- persistent pools ----------------
        s_pers = sub_off[G_PERS * GT]  # persistent one-hot subchunks
        pp = ctx.enter_context(tc.tile_pool(name="pers", bufs=1))
        xa = pp.tile([128, VP], BF16, tag="xa")
        xb = pp.tile([128, VP], BF16, tag="xb")
        oh_pers = pp.tile([128, s_pers, 128], BF16, tag="ohp")
        srcw_t = pp.tile([128, tot_sub * 8], I16, tag="srcw")
        dstloc_t = pp.tile([128, tot_sub, 1], BF16, tag="dstloc")
        iota_t = pp.tile([128, 1, 128], BF16, tag="iota")
        w0_t = pp.tile([128, 7 * 128], BF16, tag="w0")
        w1_t = pp.tile([128, 7 * 128], BF16, tag="w1")
        g0_t = pp.tile([128, 6 * 128], BF16, tag="g0")  # w0m,w1m,w0e(2),w1e(2)
        g0v_t = pp.tile([3, 256], BF16, tag="g0v")      # w0v, w1v
        offw_t = pp.tile([128, 3], BF16, tag="offw")
        ones_t = pp.tile([1, GT * 128], BF16, tag="ones")
        erow_t = pp.tile([1, 256], BF16, tag="erow")    # e0row, e1row
        encc_t = pp.tile([128, 2], BF16, tag="encc")

        nc.vector.memset(ones_t[:], 1.0)

        psA = ctx.enter_context(tc.tile_pool(name="psA", bufs=3, space="PSUM"))

        def _load_g0():
            """Layer-0 weight loads + enc rank-1 rows; issued after the first
            feature-map DMAs so they don't delay the sampling pipeline."""
            nc.sync.dma_start(g0_t[:, 0:128], ap["g0w0m"][:])
            nc.sync.dma_start(g0_t[:, 128:256], ap["g0w1m"][:])
            nc.sync.dma_start(
                g0_t[:, 256:512].rearrange("p (c h) -> p c h", h=128),
                ap["g0w0e"].rearrange("(c p) h -> p c h", p=128))
            nc.sync.dma_start(
                g0_t[:, 512:768].rearrange("p (c h) -> p c h", h=128),
                ap["g0w1e"].rearrange("(c p) h -> p c h", p=128))
            nc.sync.dma_start(g0v_t[:, 0:128], ap["g0w0v"][:])
            nc.sync.dma_start(g0v_t[:, 128:256], ap["g0w1v"][:])
            nc.sync.dma_start(offw_t[:], ap["offw"][:])
            nc.sync.dma_start(encc_t[:], ap["encc"][:])
            _load_bulk()
            # enc rank-1 rows: e{0,1} = g0_w{0,1}[131:387].T @ enc -> [1,128]
            for k in range(2):
                pe = psA.tile([1, 128], F32, tag="p1")
                for cchunk in range(2):
                    nc.tensor.matmul(
                        out=pe[:],
                        lhsT=encc_t[:, cchunk:cchunk + 1],
                        rhs=g0_t[:, 256 + k * 256 + cchunk * 128:
                                 256 + k * 256 + cchunk * 128 + 128],
                        start=(cchunk == 0), stop=(cchunk == 1))
                nc.scalar.activation(erow_t[:, k * 128:(k + 1) * 128], pe[:],
                                     AF.Copy)

        def _load_bulk():
            # bulky graph-structure loads on the Activation queue: no waits,
            # so they drain mid-sampling and fill idle DMA bandwidth without
            # head-of-line blocking the SP wsc prefetch stream
            nc.scalar.dma_start(srcw_t[:], ap["srcw"][:])
            nc.scalar.dma_start(
                dstloc_t[:], ap["dstloc"].rearrange("p (s o) -> p s o", o=1))
            nc.scalar.dma_start(iota_t[:].rearrange("p o d -> p (o d)"),
                                ap["iota"][:])
            nc.scalar.dma_start(w0_t[:], ap["gw0"][:])
            nc.scalar.dma_start(w1_t[:], ap["gw1"][:])

        def _sampling(sctx):
            """Phase 1: vert_align sampling -> xa (bf16 columns).  The layer-0
            h1 rows are produced chunk-by-chunk right after each ReLU so the
            first gathers can start as soon as sampling ends."""
            sp = sctx.enter_context(tc.tile_pool(name="samp", bufs=1))
            spfm = sctx.enter_context(tc.tile_pool(name="sampfm", bufs=2))
            spf = sctx.enter_context(tc.tile_pool(name="sampf", bufs=2))
            spw = sctx.enter_context(tc.tile_pool(name="sampw", bufs=6))
            sph = sctx.enter_context(tc.tile_pool(name="samph", bufs=3))
            spp1 = sctx.enter_context(tc.tile_pool(name="samppsum1", bufs=3,
                                                   space="PSUM"))
            spp2 = sctx.enter_context(tc.tile_pool(name="samppsum2", bufs=2,
                                                   space="PSUM"))
            g_sb = sp.tile([128, NGT * 128], BF16, tag="gsb")

            def _load_map(mi):
                C, Wm = MAPS[mi]
                ncc = C // 128
                bw_t = spf.tile([128, 16 * 128], BF16, tag="bw")
                nc.sync.dma_start(
                    bw_t[:, :ncc * 128],
                    ap["bw"][:, CH_OFF[mi]:CH_OFF[mi] + ncc * 128])
                fm_t = spfm.tile([128, 2 * 3136], BF16, tag="fm")
                nc.sync.dma_start(
                    fm_t[:, :ncc * Wm * Wm].rearrange(
                        "p (c hw) -> p c hw", c=ncc),
                    ap[f"f{mi+1}"].rearrange("(c p) hw -> p c hw", p=128))
                return fm_t, bw_t

            nxt_ld = _load_map(0)
            _load_g0()
            for mi, (C, Wm) in enumerate(MAPS):
                HW = Wm * Wm
                ncc = C // 128
                fm_t, bw_t = nxt_ld
                if mi + 1 < 4:
                    nxt_ld = _load_map(mi + 1)
                for t in range(ntile_map[mi]):
                    p0 = t * 128
                    pcnt = min(128, HW - p0)
                    pg = psA.tile([128, 128], F32, tag="p1")
                    for cc in range(ncc):
                        nc.tensor.matmul(
                            out=pg[:pcnt, :],
                            lhsT=fm_t[:, cc * HW + p0:cc * HW + p0 + pcnt],
                            rhs=bw_t[:, cc * 128:cc * 128 + 128],
                            start=(cc == 0), stop=(cc == ncc - 1))
                    gt = g_off[mi] + t
                    nc.scalar.activation(
                        g_sb[:pcnt, gt * 128:gt * 128 + 128], pg[:pcnt, :],
                        AF.Copy)

            for c in range(NVCH):
                ps = spp1.tile([128, 512], F32, tag="ps")
                pairs_c = []
                for mi in range(4):
                    for t in sched[mi][c]:
                        pairs_c.append((mi, t))
                assert len(pairs_c) == npc[c]
                half = (max_npc + 1) // 2
                nh = (npc[c] + half - 1) // half
                wts = []
                for hb in range(nh):
                    k0, k1 = hb * half, min((hb + 1) * half, npc[c])
                    wt = spw.tile([128, half, 512], FP8, tag="wsc")
                    nc.sync.dma_start(
                        wt[:, :k1 - k0, :],
                        ap["wsc"].rearrange("(k p) h -> p k h", p=128)
                        [:, npc_off[c] + k0:npc_off[c] + k1, :])
                    wts.append(wt)
                for k, (mi, t) in enumerate(pairs_c):
                    HW = MAPS[mi][1] ** 2
                    pcnt = min(128, HW - t * 128)
                    gt = g_off[mi] + t
                    nc.tensor.matmul(
                        out=ps[:],
                        lhsT=g_sb[:pcnt, gt * 128:gt * 128 + 128],
                        rhs=wts[k // half][:pcnt, k % half, :],
                        start=(k == 0), stop=(k == len(pairs_c) - 1))
                nc.scalar.activation(xa[:, c * 512:(c + 1) * 512], ps[:],
                                     AF.Relu)
                # layer-0 h1 rows for this chunk's 4 tiles
                vv = sph.tile([3, 512], BF16, tag="vt")
                nc.sync.dma_start(vv[:],
                                  ap["vertsT"][:, c * 512:(c + 1) * 512])
                ph4 = spp2.tile([128, 512], F32, tag="ph4")
                hstc = sph.tile([128, 512], BF16, tag="hstc")
                for ti in range(4):
                    t = 4 * c + ti
                    sl = slice(ti * 128, (ti + 1) * 128)
                    nc.tensor.matmul(
                        out=ph4[:, sl], lhsT=xa[:, t * 128:(t + 1) * 128],
                        rhs=g0_t[:, 128:256], start=True, stop=False)
                    nc.tensor.matmul(
                        out=ph4[:, sl], lhsT=vv[:, sl],
                        rhs=g0v_t[:, 128:256], start=False, stop=False)
                    nc.tensor.matmul(
                        out=ph4[:, sl], lhsT=ones_t[:, 0:128],
                        rhs=erow_t[:, 128:256], start=False, stop=True)
                nc.scalar.activation(hstc[:], ph4[:], AF.Copy)
                # Pool-issued so a write waiting on compute never head-of-line
                # blocks the SP queue's wsc prefetch stream.
                h1_writes.append(nc.gpsimd.dma_start(
                    h1d2[0].rearrange("(p n) c -> p n c", p=128)
                    [:, c * 4:(c + 1) * 4, :],
                    hstc[:].rearrange("p (n c) -> p n c", c=128)))

        h1_writes = []
        with ExitStack() as sctx:
            _sampling(sctx)


        # ---------------- phase 2: graph conv layers ----------------
        lp = ctx.enter_context(tc.tile_pool(name="lay", bufs=3))
        lpo = ctx.enter_context(tc.tile_pool(name="layoh", bufs=2))
        lph = ctx.enter_context(tc.tile_pool(name="layh", bufs=3))
        lpv = ctx.enter_context(tc.tile_pool(name="layv", bufs=2))
        psx = ctx.enter_context(tc.tile_pool(name="psumx", bufs=2, space="PSUM"))
        psB = ctx.enter_context(tc.tile_pool(name="psumo", bufs=1, space="PSUM"))

        def _layers(first_rep, last_rep, h1_writes):
            cur, nxt = xa, xb
            if not first_rep:
                # prologue: recompute layer-0 h1 rows (repeat mode only)
                h1_writes = []
                for g in range(NGRP):
                    hst = lph.tile([128, GT * 128], BF16, tag="hstg")
                    vv = lpv.tile([3, GT * 128], BF16, tag="vt")
                    nc.sync.dma_start(
                        vv[:], ap["vertsT"][:, g * 512:(g + 1) * 512])
                    ph4 = psx.tile([128, 512], F32, tag="ph4")
                    for ti in range(GT):
                        t = g * GT + ti
                        sl = slice(ti * 128, (ti + 1) * 128)
                        nc.tensor.matmul(
                            out=ph4[:, sl], lhsT=cur[:, t * 128:(t + 1) * 128],
                            rhs=g0_t[:, 128:256], start=True, stop=False)
                        nc.tensor.matmul(
                            out=ph4[:, sl], lhsT=vv[:, sl],
                            rhs=g0v_t[:, 128:256], start=False, stop=False)
                        nc.tensor.matmul(
                            out=ph4[:, sl], lhsT=ones_t[:, 0:128],
                            rhs=erow_t[:, 128:256], start=False, stop=True)
                    nc.scalar.activation(hst[:], ph4[:], AF.Copy)
                    h1_writes.append(nc.sync.dma_start(
                        h1d2[0].rearrange("(p n) c -> p n c", p=128)
                        [:, g * GT:(g + 1) * GT, :],
                        hst[:].rearrange("p (n c) -> p n c", c=128)))

            for l in range(nlayers):
                h1d = h1d2[l % 2]
                h1d_nxt = h1d2[(l + 1) % 2]
                next_writes = []

                # gather groups + scatter matmuls; h1 rows for layer l+1 are
                # produced group-by-group right after each ReLU so the next
                # layer's gathers can start almost immediately.
                for g in range(NGRP):
                    s0 = sub_off[g * GT]
                    s1 = sub_off[min((g + 1) * GT, NT)]
                    ng = s1 - s0
                    assert ng > 0, "empty gather group unsupported"
                    msg = lp.tile([128, sub_g_max, 128], BF16, tag="msg")
                    gi = nc.gpsimd.dma_gather(
                        out_ap=msg[:, :ng, :],
                        in_ap=h1d[:],
                        idxs_ap=srcw_t[:, s0 * 8:s1 * 8],
                        num_idxs=ng * 128,
                        num_idxs_reg=ng * 128,
                        elem_size=HID,
                        single_packet=False,
                    )
                    for wi in h1_writes:
                        tile.add_dep_helper(gi.ins, wi.ins,
                                            reason="h1 RAW: gather after write")
                    if g < G_PERS:
                        oh_t, so = oh_pers, 0
                        if l == 0 and first_rep:
                            # build the persistent one-hots (layer-invariant)
                            nc.vector.tensor_tensor(
                                out=oh_pers[:, s0:s1, :],
                                in0=dstloc_t[:, s0:s1, :]
                                .to_broadcast([128, ng, 128]),
                                in1=iota_t[:].to_broadcast([128, ng, 128]),
                                op=mybir.AluOpType.is_equal)
                    else:
                        oh_t = lpo.tile([128, sub_g_max, 128], BF16, tag="oht")
                        so = s0
                        nc.vector.tensor_tensor(
                            out=oh_t[:, :ng, :],
                            in0=dstloc_t[:, s0:s1, :]
                            .to_broadcast([128, ng, 128]),
                            in1=iota_t[:].to_broadcast([128, ng, 128]),
                            op=mybir.AluOpType.is_equal)
                    if l == 0:
                        vv2 = lpv.tile([3, GT * 128], BF16, tag="vt2")
                        nc.sync.dma_start(
                            vv2[:],
                            ap["vertsT"][:, g * GT * 128:(g + 1) * GT * 128])
                    W = GT * 128
                    px = psx.tile([128, W], F32, tag="px")
                    if l == 0:
                        nc.tensor.matmul(
                            out=px[:], lhsT=g0_t[:, 0:128],
                            rhs=cur[:, g * W:(g + 1) * W],
                            start=True, stop=False)
                        nc.tensor.matmul(
                            out=px[:], lhsT=g0v_t[:, 0:128],
                            rhs=vv2[:], start=False, stop=False)
                        nc.tensor.matmul(
                            out=px[:], lhsT=erow_t[:, 0:128],
                            rhs=ones_t[:], start=False, stop=False)
                    else:
                        nc.tensor.matmul(
                            out=px[:], lhsT=w0_t[:, (l - 1) * 128:l * 128],
                            rhs=cur[:, g * W:(g + 1) * W],
                            start=True, stop=False)
                    live = [(ti, j) for ti in range(GT)
                            for j in range(nsub_t[g * GT + ti])]
                    for k, (ti, j) in enumerate(live):
                        t = g * GT + ti
                        s = sub_off[t] - s0 + j
                        nc.tensor.matmul(
                            out=px[:, ti * 128:(ti + 1) * 128],
                            lhsT=msg[:, s, :],
                            rhs=oh_t[:, sub_off[t] + j - so, :],
                            start=False,
                            stop=(k == len(live) - 1),
                            skip_group_check=True)
                    nc.scalar.activation(nxt[:, g * W:(g + 1) * W], px[:],
                                         AF.Relu)
                    if l == nlayers - 1 and last_rep:
                        # delta_v for this group: off_w.T @ x cols -> [3, 512]
                        po = psB.tile([3, GT * 128], F32, tag="po")
                        nc.tensor.matmul(
                            out=po[:], lhsT=offw_t[:],
                            rhs=nxt[:, g * W:(g + 1) * W],
                            start=True, stop=True)
                        ost = lph.tile([3, GT * 128], F32, tag="ost")
                        nc.scalar.activation(ost[:], po[:], AF.Copy)
                        nc.sync.dma_start(out[:, g * W:(g + 1) * W], ost[:])
                    if l + 1 < nlayers:
                        # h1 rows for layer l+1 on this group's tiles
                        hst = lph.tile([128, GT * 128], BF16, tag="hstg")
                        ph4 = psx.tile([128, 512], F32, tag="ph4")
                        for ti in range(GT):
                            t = g * GT + ti
                            nc.tensor.matmul(
                                out=ph4[:, ti * 128:(ti + 1) * 128],
                                lhsT=nxt[:, t * 128:(t + 1) * 128],
                                rhs=w1_t[:, l * 128:(l + 1) * 128],
                                start=True, stop=True)
                        nc.scalar.activation(hst[:], ph4[:], AF.Copy)
                        next_writes.append(nc.sync.dma_start(
                            h1d_nxt.rearrange("(p n) c -> p n c", p=128)
                            [:, g * GT:(g + 1) * GT, :],
                            hst[:].rearrange("p (n c) -> p n c", c=128)))
                h1_writes = next_writes
                cur, nxt = nxt, cur

        for _rep in range(repeat):
            _layers(_rep == 0, _rep == repeat - 1, h1_writes)
        cur = xa if nlayers % 2 == 0 else xb

        if xdump is not None:
            nc.sync.dma_start(xdump[:], cur[:])

        if nlayers == 0:
            # output straight from the sampled activations (debug path)
            for g in range(NGRP):
                po = psB.tile([3, GT * 128], F32, tag="po")
                nc.tensor.matmul(
                    out=po[:], lhsT=offw_t[:],
                    rhs=cur[:, g * 512:(g + 1) * 512], start=True, stop=True)
                ost = lph.tile([3, GT * 128], F32, tag="ost")
                nc.scalar.activation(ost[:], po[:], AF.Copy)
                nc.sync.dma_start(out[:, g * 512:(g + 1) * 512], ost[:])

    nc.compile()
    return nc


_CACHE = {}


def kernel(**inputs) -> np.ndarray:
    cfg, per_core, post = _prep(inputs)
    key = (cfg["npair"], tuple(cfg["npc"]), cfg["tot_sub"],
           tuple(cfg["nsub_t"]))
    if key not in _CACHE:
        _CACHE[key] = _build(cfg, per_core[0])
    nc = _CACHE[key]
    res = run_bass_kernel_spmd(nc, per_core, list(range(B)))
    outs = np.empty((B, V, 3), np.float32)
    for m in range(B):
        rows = np.ascontiguousarray(res.results[m]["out"].T)[:V]
        outs[m][post["sigmas"][m]] = rows
    return outs.reshape(B * V, 3)


if __name__ == "__main__":
    pass
